# revision 12
# baseline (speedup 1.0000x reference)
"""Trainium2 Bass kernel for nn_AMIPRouterInference (gnn_message_passing).

Strategy
--------
Algebraic restructure of the reference (~515 GFLOP -> ~52 GFLOP):
  * cond @ W1 splits into h_anc @ W1a + h_ctr @ W1b, each computed once per
    token (not once per window pair):  u = h @ W1b, v = h @ W1a.
  * The attention combine over the +-r window commutes with the W2 matmul:
    g = sum_n cw_n * gelu(v[l+off_n] + u[l]);  delta = (w * g) @ W2 + w @ b2.

Sharding: pure data-parallel over the B*L = 4096 tokens -> 512 tokens/core on
8 cores; the +-5 halo is baked into each core's input shard on the host, so no
collectives are needed.

Per-core layout: features-on-partitions (u/v as 16 chunks of [128, tokens]) so
window shifts along tokens are free-axis SBUF slices.  Even/odd phase copies of
v keep the bf16 DVE 2x alignment for shifted adds.

Key engine facts this schedule is built around:
  * DVE is the bottleneck engine (~165us of tensor_tensor at bf16 2x).
    Batched multi-row-AP adds keep the 2x packing when every row start is
    4B-aligned (hardware-verified).
  * PE clock is HAM-gated: 1.2 GHz cold, 2.4 GHz after ~3.4us of sustained
    activity; any >3.4us idle window re-throttles.  The delta-stage matmuls
    are paced per-combine through the back half so the post-combine(15)
    tail is only ~23 matmuls.
  * DMA issue costs ~650ns per descriptor on the in-order sync queue, so
    startup inputs are packed host-side into 5 large contiguous transfers.
  * A 4-fc emission runway (d_mm 0..3 before combine 0) gives the DVE queue
    adds-work to chew while the exp->broadcast round trip for cw lands;
    combines then trail d_mm by 4 fc for the rest of the kernel, which also
    keeps ACT's gelu well ahead of the combine that consumes it.
"""

import sys

for _p in ("/opt/trn_rl_repo", "/root/.axon_site/_ro/trn_rl_repo"):
    if _p not in sys.path:
        sys.path.append(_p)

import numpy as np
import ml_dtypes

import bass_rust
import concourse.bacc as bacc
import concourse.mybir as mybir
import concourse.tile as tile
from concourse.bass_utils import run_bass_kernel_spmd

BF16 = ml_dtypes.bfloat16

# Problem constants (hardcoded per spec).
B, L, D = 2, 2048, 1024
K, D4, R = 8, 256, 5
NCORES = 8
T = (B * L) // NCORES          # tokens per core = 512
PADL = 16                      # left pad of the per-core token window
TP = T + 2 * PADL              # padded width = 544
NOFF = 2 * R                   # 10 window offsets
F = K * D4                     # 2048 fused expert features
NFC = F // 128                 # 16 feature chunks
NKC = D // 128                 # 8 contraction chunks
NTC = T // 128                 # 4 token tiles per core

# Offset processing order: even offsets first (read from v_even), then odd
# (read from v_odd, which holds v shifted left by one token).  Within each
# phase every slice start is an even element index -> 4-byte aligned, which
# keeps the DVE's bf16 2x packing for the batched multi-row adds.
OFF_ORDER = [-4, -2, 2, 4, -5, -3, -1, 1, 3, 5]

RUNWAY = 4                     # d_mm emitted this many fc ahead of combine

_SIM_SAFE_GELU = False         # CoreSim lacks Gelu; swap in Tanh for sim runs

_CACHE = {}


def _build_graph():
    fp32 = mybir.dt.float32
    bf16 = mybir.dt.bfloat16

    nc = bacc.Bacc("TRN2", target_bir_lowering=False, debug=False,
                   num_devices=NCORES)

    # ---- DRAM parameters (per-core shards; same shapes on every core).
    # Startup tensors are host-packed so each is ONE contiguous DMA.
    hP = nc.dram_tensor("hP", [128, NKC * TP], bf16, kind="ExternalInput")
    cP = nc.dram_tensor("cP", [128, 176], fp32, kind="ExternalInput")
    wrP = nc.dram_tensor("wrP", [128, NKC * K], bf16, kind="ExternalInput")
    validT = nc.dram_tensor("validT", [NOFF, T], fp32, kind="ExternalInput")
    b2o = nc.dram_tensor("b2o", [NOFF, D + 2], bf16, kind="ExternalInput")
    w1ab = nc.dram_tensor("w1ab", [NFC, 128, 2 * D], bf16,
                          kind="ExternalInput")
    w2 = nc.dram_tensor("w2", [NFC, 128, D], bf16, kind="ExternalInput")
    out = nc.dram_tensor("out", [T, D], fp32, kind="ExternalOutput")

    AF = mybir.ActivationFunctionType
    OP = mybir.AluOpType

    def bc_ap(tile_, inner_rep, ncols):
        """[128, ncols] tile viewed as [128, ncols, inner_rep] via a step-0
        innermost dim (per-partition broadcast along the replicated axis)."""
        return bass_rust.AP(
            tensor=tile_[:].tensor, offset=0,
            ap=[[ncols, 128], [1, ncols], [0, inner_rep]])

    def rows_ap(tile_, off, ostep, ocnt, icnt):
        """Multi-row free AP: ocnt rows of icnt step-1 elements, row starts
        off, off+ostep, ...  (all starts must be 4B-aligned for bf16 2x)."""
        return bass_rust.AP(
            tensor=tile_[:].tensor, offset=off,
            ap=[[tile_[:].shape[1], 128], [ostep, ocnt], [1, icnt]])

    with tile.TileContext(nc) as tc:
        with (
            tc.tile_pool(name="const", bufs=1) as cpool,
            tc.tile_pool(name="hpool", bufs=1) as hpool,
            tc.tile_pool(name="w2pool", bufs=1) as w2pool,
            tc.tile_pool(name="w1pool", bufs=4) as w1pool,
            tc.tile_pool(name="small", bufs=2) as spool,
            tc.tile_pool(name="persist", bufs=1) as ppool,
            tc.tile_pool(name="uv", bufs=4) as uvpool,
            tc.tile_pool(name="big", bufs=RUNWAY + 1) as bigpool,
            tc.tile_pool(name="qbuf", bufs=1) as q1pool,
            tc.tile_pool(name="tbuf", bufs=1) as qpool,
            tc.tile_pool(name="ppart", bufs=1) as partpool,
            tc.tile_pool(name="gout", bufs=1) as gpool,
            tc.tile_pool(name="opool", bufs=4) as opool,
            tc.tile_pool(name="dram", bufs=1, space="DRAM") as dpool,
            tc.tile_pool(name="ps_big", bufs=4, space="PSUM") as psb,
            tc.tile_pool(name="ps_vb", bufs=1, space="PSUM") as psvb,
            tc.tile_pool(name="ps_small", bufs=3, space="PSUM") as pss,
            # PSUM budget (8 banks): psb "m" 4 (u/va double buffer; the 4
            # banks are reused for delta preopens once stage D ends),
            # psvb "vb" 1, pss "s" 3 (logits, transposes, den, E1
            # transients, and the 3 held delta groups).
        ):
            # ---------------- packed startup loads ----------------
            h_m = hpool.tile([128, NKC * TP], bf16, tag="h")
            nc.sync.dma_start(h_m[:], hP[:])
            c_m = cpool.tile([128, 176], fp32, tag="c")
            nc.sync.dma_start(c_m[:], cP[:])
            wr_m = cpool.tile([128, NKC * K], bf16, tag="wr")
            nc.sync.dma_start(wr_m[:], wrP[:])
            validT_sb = cpool.tile([NOFF, T], fp32, tag="validT")
            nc.sync.dma_start(validT_sb[:], validT[:])
            b2o_sb = cpool.tile([NOFF, D + 2], bf16, tag="b2o")
            nc.sync.dma_start(b2o_sb[:], b2o[:])

            h_sb = [h_m[:, kc * TP:(kc + 1) * TP] for kc in range(NKC)]
            ident_sb = c_m[:, 0:128]
            br_sb = c_m[:, 128:160]
            b1_sb = c_m[:, 160:176]
            wr_sb = [wr_m[:, kc * K:(kc + 1) * K] for kc in range(NKC)]
            b2_sb = b2o_sb[0:K, 0:D]
            ones10_sb = b2o_sb[:, D:D + 1]

            # fc0's W1 rides the (idle) Tensor-engine DMA queue so it
            # lands in parallel with the sync queue's h/const loads.
            w1_first = w1pool.tile([128, 2 * D], bf16, tag="w1")
            nc.scalar.dma_start(w1_first[:], w1ab[0])

            # Hoist both ACT table loads (exp + gelu sets, ~1.3us each) into
            # the h-DMA shadow via 1-column dummy activations.
            warm = spool.tile([1, 1], fp32, tag="warm")
            nc.scalar.activation(warm[:], ident_sb[0:1, 0:1], AF.Exp)
            nc.scalar.activation(warm[:], warm[:], AF.Tanh if _SIM_SAFE_GELU else AF.Gelu)

            # persistent transposed score & router weights (bf16)
            cwT_bf = ppool.tile([NOFF, T], bf16, tag="cwT")
            wT_bf = ppool.tile([K, T], bf16, tag="wT")
            cw_bc = gpool.tile([128, NOFF * 512], bf16, tag="cw_bc")
            w_bc_all = gpool.tile([128, K * 512], bf16, tag="w_bc_all")

            # ------------- stage A/B/C: scores, cw, router w -------------
            # Phase A: gram/router matmuls; each tile's diagonal extraction
            # is emitted right behind its gram so DVE starts ASAP; the logit
            # evac follows immediately so only one "s" bank is held per tile.
            s_all = spool.tile([128, NTC * NOFF], fp32, tag="s_all")
            junk = spool.tile([128, 128], fp32, tag="junk")
            lg_all = spool.tile([128, NTC * K], fp32, tag="lg_all")

            def phase_gram(tci):
                c0 = PADL + tci * 128
                g_ps = psb.tile([128, 512], fp32, tag="m")
                lg_ps = pss.tile([128, K], fp32, tag="s")
                for kc in range(NKC):
                    st = (kc == 0)
                    sp = (kc == NKC - 1)
                    nc.tensor.matmul(g_ps[:, :138],
                                     h_sb[kc][:, c0:c0 + 128],
                                     h_sb[kc][:, c0 - 5:c0 + 133],
                                     start=st, stop=sp)
                    nc.tensor.matmul(lg_ps[:],
                                     h_sb[kc][:, c0:c0 + 128],
                                     wr_sb[kc],
                                     start=st, stop=sp)
                for n, off in enumerate(OFF_ORDER):
                    nc.vector.affine_mul_reduce(
                        junk[:], s_all[:, tci * NOFF + n:tci * NOFF + n + 1],
                        g_ps[:, off + 5:off + 5 + 128], ident_sb,
                        1.0 / 32.0, 0.0)
                nc.scalar.copy(lg_all[:, tci * K:(tci + 1) * K], lg_ps[:])

            # Phase B1: transpose scores to [NOFF, T], then broadcast the
            # UNNORMALIZED ev = exp(s)*valid right away.  The softmax 1/den
            # lands at the very end as a per-token (=per-partition) scale on
            # the delta close.  No max-shift is needed: scores are O(1) so
            # exp() cannot over/underflow.
            sT = ppool.tile([NOFF, T], fp32, tag="sT")

            def phase_b1():
                for tci in range(NTC):
                    sT_ps = pss.tile([NOFF, 128], fp32, tag="s")
                    nc.tensor.transpose(sT_ps[:],
                                        s_all[:, tci * NOFF:(tci + 1) * NOFF],
                                        ident_sb)
                    nc.scalar.copy(sT[:, tci * 128:(tci + 1) * 128], sT_ps[:])
                evT = ppool.tile([NOFF, T], fp32, tag="evT")
                nc.scalar.activation(evT[:], sT[:], AF.Exp)
                nc.vector.tensor_mul(cwT_bf[:], evT[:], validT_sb[:])
                cw_dram = dpool.tile([1, NOFF * T], bf16, tag="cw_dram")
                nc.scalar.dma_start(cw_dram[:], cwT_bf[:])
                nc.scalar.dma_start(cw_bc[:],
                                    cw_dram[:].partition_broadcast(128))

            rdenT = ppool.tile([128, NTC], fp32, tag="rdenT")
            fT = ppool.tile([128, NTC], fp32, tag="fT")
            wplT_bf = ppool.tile([K, T], bf16, tag="wplT")
            w_pl = ppool.tile([128, NTC * K], fp32, tag="w_pl")

            we = spool.tile([128, NTC * K], fp32, tag="we")

            def phase_b2a_pre():
                """Router softmax up to the exp."""
                nc.vector.tensor_add(lg_all[:], lg_all[:], br_sb)
                wmx = spool.tile([128, NTC], fp32, tag="wmx")
                lg3 = bass_rust.AP(tensor=lg_all[:].tensor, offset=0,
                                   ap=[[NTC * K, 128], [K, NTC], [1, K]])
                nc.vector.reduce_max(wmx[:], lg3, mybir.AxisListType.X)
                nc.vector.tensor_sub(we[:], lg_all[:], bc_ap(wmx, K, NTC))
                nc.scalar.activation(we[:], we[:], AF.Exp)

            def phase_b2a_post():
                """Post-exp half: w softmax, transpose, broadcast.  Emitted
                before the first gelu so the w_dram round trip isn't queued
                behind 5.5us ACT gelus."""
                wsum = spool.tile([128, NTC], fp32, tag="wsum")
                we3 = bass_rust.AP(tensor=we[:].tensor, offset=0,
                                   ap=[[NTC * K, 128], [K, NTC], [1, K]])
                nc.vector.reduce_sum(wsum[:], we3, mybir.AxisListType.X)
                rws = spool.tile([128, NTC], fp32, tag="rws")
                nc.vector.reciprocal(rws[:], wsum[:])
                nc.vector.tensor_mul(w_pl[:], we[:], bc_ap(rws, K, NTC))
                for tci in range(NTC):
                    wpT_ps = pss.tile([K, 128], fp32, tag="s")
                    nc.tensor.transpose(wpT_ps[:],
                                        w_pl[:, tci * K:(tci + 1) * K],
                                        ident_sb)
                    nc.scalar.copy(wplT_bf[:, tci * 128:(tci + 1) * 128],
                                   wpT_ps[:])
                w_dram = dpool.tile([1, K * T], bf16, tag="w_dram")
                nc.scalar.dma_start(w_dram[:], wplT_bf[:])
                nc.scalar.dma_start(w_bc_all[:],
                                    w_dram[:].partition_broadcast(128))

            def phase_b2b():
                """Denominators + per-token scale columns + b2-path weights.
                The reciprocal runs on the [128, NTC] transposed layout -- a
                [1, T] reciprocal would serialize 512 8-cycle divides on one
                partition (~4us)."""
                den_ps = pss.tile([1, T], fp32, tag="s")
                nc.tensor.matmul(den_ps[:], ones10_sb, cwT_bf[:],
                                 start=True, stop=True)
                den = ppool.tile([1, T], fp32, tag="den")
                nc.scalar.copy(den[:], den_ps[:])       # raw sum_n ev
                for tci in range(NTC):
                    nc.scalar.dma_start(fT[:, tci:tci + 1],
                                        den[:, tci * 128:(tci + 1) * 128])
                dene_t = ppool.tile([128, NTC], fp32, tag="dene_t")
                nc.vector.tensor_scalar_add(dene_t[:], fT[:], 1e-30)
                nc.vector.reciprocal(rdenT[:], dene_t[:])
                # b2-path weights: w * raw_den (so the final 1/den scale on
                # the delta close reproduces w * sum_cw exactly)
                weff3 = spool.tile([128, NTC * K], fp32, tag="weff3")
                for tci in range(NTC):
                    nc.vector.tensor_scalar_mul(
                        weff3[:, tci * K:(tci + 1) * K],
                        w_pl[:, tci * K:(tci + 1) * K], fT[:, tci:tci + 1])
                for tci in range(NTC):
                    weT_ps = pss.tile([K, 128], fp32, tag="s")
                    nc.tensor.transpose(weT_ps[:],
                                        weff3[:, tci * K:(tci + 1) * K],
                                        ident_sb)
                    nc.scalar.copy(wT_bf[:, tci * 128:(tci + 1) * 128],
                                   weT_ps[:])

            # ------------- stage D: u/v matmuls + gelu combine -------------
            g_sb = [None] * NFC
            tmp_sb = [None] * NFC

            def stage_d_mm(fc, w1_pre=None):
                if w1_pre is None:
                    w1_t = w1pool.tile([128, 2 * D], bf16, tag="w1")
                    nc.sync.dma_start(w1_t[:], w1ab[fc])
                else:
                    w1_t = w1_pre

                u_ps = psb.tile([128, 512], fp32, tag="m")
                va_ps = psb.tile([128, 512], fp32, tag="m")
                vb_ps = psvb.tile([128, 48], fp32, tag="vb")
                for kc in range(NKC):
                    st = (kc == 0)
                    sp = (kc == NKC - 1)
                    lhs_b = w1_t[:, kc * 128:(kc + 1) * 128]
                    lhs_a = w1_t[:, D + kc * 128:D + (kc + 1) * 128]
                    nc.tensor.matmul(u_ps[:], lhs_b,
                                     h_sb[kc][:, PADL:PADL + 512],
                                     start=st, stop=sp)
                    nc.tensor.matmul(va_ps[:], lhs_a,
                                     h_sb[kc][:, 0:512],
                                     start=st, stop=sp)
                    nc.tensor.matmul(vb_ps[:], lhs_a,
                                     h_sb[kc][:, 496:544],
                                     start=st, stop=sp)

                u_sb = uvpool.tile([128, 512], bf16, tag="u")
                nc.scalar.copy(u_sb[:], u_ps[:])
                v_ev = uvpool.tile([128, TP], bf16, tag="v_ev")
                nc.scalar.copy(v_ev[:, 0:512], va_ps[:])
                nc.scalar.copy(v_ev[:, 512:544], vb_ps[:, 16:48])
                # odd phase built straight from PSUM (keeps DMA out of the
                # critical chain)
                v_od = uvpool.tile([128, TP], bf16, tag="v_od")
                nc.scalar.copy(v_od[:, 0:511], va_ps[:, 1:512])
                nc.scalar.copy(v_od[:, 511:543], vb_ps[:, 16:48])

                tmp = bigpool.tile([128, NOFF * 512], bf16, tag="tmp")
                # Batched shifted adds: every row start is an even element
                # index (4B-aligned), so the multi-row APs keep the DVE's
                # bf16 2x packing (hardware-verified: 6-row 1752ns vs
                # 6x418ns single-row).  Layout matches OFF_ORDER:
                #   [0:1024)    offs -4,-2    from v_ev
                #   [1024:2048) offs 2,4      from v_ev
                #   [2048:5120) offs -5..5 odd from v_od
                nc.vector.tensor_add(
                    rows_ap(tmp, 0, 512, 2, 512),
                    rows_ap(v_ev, PADL - 4, 2, 2, 512),
                    rows_ap(u_sb, 0, 0, 2, 512))
                nc.vector.tensor_add(
                    rows_ap(tmp, 1024, 512, 2, 512),
                    rows_ap(v_ev, PADL + 2, 2, 2, 512),
                    rows_ap(u_sb, 0, 0, 2, 512))
                nc.vector.tensor_add(
                    rows_ap(tmp, 2048, 512, 6, 512),
                    rows_ap(v_od, PADL - 1 - 5, 2, 6, 512),
                    rows_ap(u_sb, 0, 0, 6, 512))
                nc.scalar.activation(tmp[:], tmp[:], AF.Tanh if _SIM_SAFE_GELU else AF.Gelu,
                                     bias=b1_sb[:, fc:fc + 1])
                tmp_sb[fc] = tmp

            def stage_d_combine(fc):
                tmp = tmp_sb[fc]
                q = q1pool.tile([128, NOFF * 512], bf16, tag="q")
                nc.vector.tensor_mul(q[:], tmp[:], cw_bc[:])

                # pairwise tree-sum of the 10 weighted slices, then w-scale
                t1 = qpool.tile([128, 2560], bf16, tag="t1")
                nc.vector.tensor_add(t1[:], q[:, 0:2560], q[:, 2560:5120])
                t2 = qpool.tile([128, 1024], bf16, tag="t2")
                nc.vector.tensor_add(t2[:], t1[:, 0:1024], t1[:, 1024:2048])
                t3 = qpool.tile([128, 512], bf16, tag="t3")
                nc.vector.tensor_add(t3[:], t2[:, 0:512], t2[:, 512:1024])
                t4 = qpool.tile([128, 512], bf16, tag="t4")
                nc.vector.tensor_add(t4[:], t3[:], t1[:, 2048:2560])
                g_t = gpool.tile([128, 512], bf16, tag=f"g{fc}")
                nc.vector.tensor_mul(
                    g_t[:], t4[:],
                    w_bc_all[:, (fc // 2) * 512:(fc // 2) * 512 + 512])
                g_sb[fc] = g_t

            w2_sb = [None] * NFC

            def load_w2(j):
                t = w2pool.tile([128, D], bf16, tag=f"w2_{j}")
                nc.sync.dma_start(t[:], w2[j])
                w2_sb[j] = t

            def blk_mm(d_ps, blk, fc, start, stop=False):
                tci, dh = blk // 2, blk % 2
                nc.tensor.matmul(
                    d_ps[:],
                    g_sb[fc][:, tci * 128:(tci + 1) * 128],
                    w2_sb[fc][:, dh * 512:(dh + 1) * 512],
                    start=start, stop=stop)

            def blk_b2_mm(d_ps, blk):
                tci, dh = blk // 2, blk % 2
                nc.tensor.matmul(
                    d_ps[:],
                    wT_bf[:, tci * 128:(tci + 1) * 128],
                    b2_sb[:, dh * 512:(dh + 1) * 512],
                    start=False, stop=True)

            def out_dma(o_sb, blk):
                tci, dh = blk // 2, blk % 2
                nc.sync.dma_start(
                    out[tci * 128:(tci + 1) * 128,
                        dh * 512:(dh + 1) * 512], o_sb[:])

            # -- delta groups.  Held groups (blocks 0..2 on "s" banks, and
            # preopened blocks 3..6 on freed "m" banks) accumulate fc matmuls
            # per combine and close with b2 + a per-token 1/den scale.
            open_ps = {}

            def grp_open(blk, g_lo, g_hi, pool):
                d_ps = pool.tile([128, 512],
                                 mybir.dt.float32, tag="m" if pool is psb
                                 else "s", name=f"dps{blk}")
                for fc in range(g_lo, g_hi + 1):
                    blk_mm(d_ps, blk, fc, start=(fc == g_lo))
                open_ps[blk] = d_ps

            def grp_extend(blk, fc):
                blk_mm(open_ps[blk], blk, fc, start=False)

            def grp_close_direct(blk):
                """For groups that accumulated all of fc 0..15."""
                tci = blk // 2
                d_ps = open_ps[blk]
                blk_b2_mm(d_ps, blk)
                o_sb = opool.tile([128, 512], fp32, tag="o")
                nc.scalar.mul(o_sb[:], d_ps[:], rdenT[:, tci:tci + 1])
                out_dma(o_sb, blk)

            d_part = {}

            def stage_e1(blk):
                """fc 0..7 partial for blocks 3..7 (one pss bank transient)."""
                tci = blk // 2
                d_ps = pss.tile([128, 512], fp32, tag="s")
                for fc in range(8):
                    blk_mm(d_ps, blk, fc, start=(fc == 0), stop=(fc == 7))
                p_t = partpool.tile([128, 512], bf16, tag=f"p{blk}")
                nc.scalar.mul(p_t[:], d_ps[:], rdenT[:, tci:tci + 1])
                d_part[blk] = p_t

            def grp_close_merge(blk):
                """For groups that accumulated fc 8..15: merge with the E1
                partial via one scalar_tensor_tensor."""
                tci = blk // 2
                d_ps = open_ps[blk]
                blk_b2_mm(d_ps, blk)
                o_sb = opool.tile([128, 512], fp32, tag="o")
                nc.vector.scalar_tensor_tensor(
                    o_sb[:], d_ps[:], rdenT[:, tci:tci + 1], d_part[blk][:],
                    op0=OP.mult, op1=OP.add)
                out_dma(o_sb, blk)

            # ---- emission schedule ----
            phase_gram(0)
            phase_gram(1)
            stage_d_mm(0, w1_pre=w1_first)
            phase_gram(2)
            phase_gram(3)
            phase_b1()
            phase_b2a_pre()
            phase_b2a_post()
            phase_b2b()
            for fc in range(1, RUNWAY):
                stage_d_mm(fc)
            for j in range(NFC):            # combine index
                jj = j + RUNWAY
                if jj < NFC:
                    stage_d_mm(jj)
                    if 4 <= jj <= 11:
                        load_w2(2 * (jj - 4))
                        load_w2(2 * (jj - 4) + 1)
                stage_d_combine(j)
                if 7 <= j <= 10:
                    stage_e1(j - 4)         # blocks 3..6
                if j == 11:
                    stage_e1(7)
                    grp_open(0, 0, 11, pss)         # held, g0..11
                if j == 12:
                    grp_extend(0, 12)
                    grp_open(1, 0, 12, pss)         # held, g0..12
                    for blk in range(3, 7):         # preopens on freed m
                        grp_open(blk, 8, 12, psb)
                if j == 13:
                    for blk in (0, 1, 3, 4, 5, 6):
                        grp_extend(blk, 13)
                    grp_open(2, 0, 13, pss)         # held, g0..13
                if j == 14:
                    for blk in (0, 1, 2, 3, 4, 5, 6):
                        grp_extend(blk, 14)
            # tail: one g15 + b2 per open group, then block 7 full
            for blk in (0, 1, 2, 3, 4, 5, 6):
                grp_extend(blk, 15)
            grp_close_direct(0)
            grp_open(7, 8, 15, pss)
            grp_close_direct(1)
            grp_close_direct(2)
            for blk in (3, 4, 5, 6):
                grp_close_merge(blk)
            grp_close_merge(7)

    nc.compile()
    return nc


def _prep_shards(h_L, mask_flags, Wr, br, W1, b1, W2, b2):
    """Host-side shard construction (numpy only; cheap vs device work)."""
    f32 = np.float32
    h_L = np.asarray(h_L, f32)
    mask = np.asarray(mask_flags)
    Wr = np.asarray(Wr, f32)
    W1 = np.asarray(W1, f32)
    W2 = np.asarray(W2, f32)
    br = np.asarray(br, f32)
    b1 = np.asarray(b1, f32)
    b2 = np.asarray(b2, f32)

    # shared (replicated) weight blocks
    w1a = np.ascontiguousarray(
        W1[:, :D, :].transpose(1, 0, 2).reshape(D, F)
        .reshape(NKC, 128, NFC, 128).transpose(2, 1, 0, 3)
        .reshape(NFC, 128, D)).astype(BF16)
    w1b = np.ascontiguousarray(
        W1[:, D:, :].transpose(1, 0, 2).reshape(D, F)
        .reshape(NKC, 128, NFC, 128).transpose(2, 1, 0, 3)
        .reshape(NFC, 128, D)).astype(BF16)
    w1ab = np.concatenate([w1b, w1a], axis=2)        # [NFC, 128, 2D]
    w2p = np.ascontiguousarray(
        W2.reshape(F, D).reshape(NFC, 128, D)).astype(BF16)
    # packed wr: [128, NKC*K]
    wrP = np.ascontiguousarray(
        Wr.reshape(NKC, 128, K).transpose(1, 0, 2).reshape(128, NKC * K)
    ).astype(BF16)
    # packed consts: ident | br_bc | b1s  -> [128, 176] fp32
    br_bc = np.tile(np.broadcast_to(br[None, :], (128, K)), (1, NTC)).astype(f32)
    b1s = np.ascontiguousarray(b1.reshape(F).reshape(NFC, 128).T)
    cP = np.concatenate([np.eye(128, dtype=f32), br_bc, b1s], axis=1)
    # packed b2 + ones column: [NOFF, D+1] bf16
    b2o = np.zeros((NOFF, D + 2), BF16)
    b2o[:K, :D] = b2.astype(BF16)
    b2o[:, D] = 1.0

    offs = np.array(OFF_ORDER, np.int64)
    in_maps = []
    outs_meta = []
    per_batch = L // (NCORES // B)          # 512 tokens, 4 shards per batch
    for c in range(NCORES):
        b = c // (NCORES // B)
        t0 = (c % (NCORES // B)) * per_batch
        # padded, transposed h slice  [D, TP] -> packed [128, NKC*TP]
        hpad = np.zeros((TP, D), f32)
        lo = t0 - PADL
        hi = t0 + T + PADL
        slo, shi = max(lo, 0), min(hi, L)
        hpad[slo - lo:shi - lo] = h_L[b, slo:shi]
        hTa = np.ascontiguousarray(hpad.T).astype(BF16)          # [D, TP]
        hP = np.ascontiguousarray(
            hTa.reshape(NKC, 128, TP).transpose(1, 0, 2)
            .reshape(128, NKC * TP))

        # validity per (token, offset-order) -> [NOFF, T]
        tok = t0 + np.arange(T)
        nbr = tok[:, None] + offs[None, :]
        inb = (nbr >= 0) & (nbr < L)
        nbrc = np.clip(nbr, 0, L - 1)
        is_m = (mask[b] == 1)
        val = (inb & is_m[tok][:, None] & (~is_m[nbrc])).astype(f32)
        valT = np.ascontiguousarray(val.T)            # [NOFF, T]
        in_maps.append({
            "hP": hP, "cP": cP, "wrP": wrP, "validT": valT, "b2o": b2o,
            "w1ab": w1ab, "w2": w2p,
        })
        outs_meta.append((b, t0))
    return in_maps, outs_meta


def kernel(**inputs):
    assert int(inputs["range_r"]) == R
    if "nc" not in _CACHE:
        _CACHE["nc"] = _build_graph()
    nc = _CACHE["nc"]
    in_maps, outs_meta = _prep_shards(
        inputs["h_L"], inputs["mask_flags"], inputs["Wr"], inputs["br"],
        inputs["W1"], inputs["b1"], inputs["W2"], inputs["b2"])
    res = run_bass_kernel_spmd(nc, in_maps, core_ids=list(range(NCORES)))
    out = np.zeros((B, L, D), np.float32)
    for c, (b, t0) in enumerate(outs_meta):
        out[b, t0:t0 + T] = res.results[c]["out"]
    return out


# revision 13
# speedup vs baseline: 1.0000x; 1.0000x over previous
"""Trainium2 Bass kernel for nn_AMIPRouterInference (gnn_message_passing).

Strategy
--------
Algebraic restructure of the reference (~515 GFLOP -> ~52 GFLOP):
  * cond @ W1 splits into h_anc @ W1a + h_ctr @ W1b, each computed once per
    token (not once per window pair):  u = h @ W1b, v = h @ W1a.
  * The attention combine over the +-r window commutes with the W2 matmul:
    g = sum_n cw_n * gelu(v[l+off_n] + u[l]);  delta = (w * g) @ W2 + w @ b2.

Sharding: pure data-parallel over the B*L = 4096 tokens -> 512 tokens/core on
8 cores; the +-5 halo is baked into each core's input shard on the host, so no
collectives are needed.

Per-core layout: features-on-partitions (u/v as 16 chunks of [128, tokens]) so
window shifts along tokens are free-axis SBUF slices.  Even/odd phase copies of
v keep the bf16 DVE 2x alignment for shifted adds.

Key engine facts this schedule is built around:
  * DVE is the bottleneck engine (~165us of tensor_tensor at bf16 2x).
    Batched multi-row-AP adds keep the 2x packing when every row start is
    4B-aligned (hardware-verified).
  * PE clock is HAM-gated: 1.2 GHz cold, 2.4 GHz after ~3.4us of sustained
    activity; any >3.4us idle window re-throttles.  The delta-stage matmuls
    are paced per-combine through the back half so the post-combine(15)
    tail is only ~23 matmuls.
  * DMA issue costs ~650ns per descriptor on the in-order sync queue, so
    startup inputs are packed host-side into 5 large contiguous transfers.
  * A 4-fc emission runway (d_mm 0..3 before combine 0) gives the DVE queue
    adds-work to chew while the exp->broadcast round trip for cw lands;
    combines then trail d_mm by 4 fc for the rest of the kernel, which also
    keeps ACT's gelu well ahead of the combine that consumes it.
"""

import sys

for _p in ("/opt/trn_rl_repo", "/root/.axon_site/_ro/trn_rl_repo"):
    if _p not in sys.path:
        sys.path.append(_p)

import numpy as np
import ml_dtypes

import bass_rust
import concourse.bacc as bacc
import concourse.mybir as mybir
import concourse.tile as tile
from concourse.bass_utils import run_bass_kernel_spmd

BF16 = ml_dtypes.bfloat16

# Problem constants (hardcoded per spec).
B, L, D = 2, 2048, 1024
K, D4, R = 8, 256, 5
NCORES = 8
T = (B * L) // NCORES          # tokens per core = 512
PADL = 16                      # left pad of the per-core token window
TP = T + 2 * PADL              # padded width = 544
NOFF = 2 * R                   # 10 window offsets
F = K * D4                     # 2048 fused expert features
NFC = F // 128                 # 16 feature chunks
NKC = D // 128                 # 8 contraction chunks
NTC = T // 128                 # 4 token tiles per core

# Offset processing order: even offsets first (read from v_even), then odd
# (read from v_odd, which holds v shifted left by one token).  Within each
# phase every slice start is an even element index -> 4-byte aligned, which
# keeps the DVE's bf16 2x packing for the batched multi-row adds.
OFF_ORDER = [-4, -2, 2, 4, -5, -3, -1, 1, 3, 5]

RUNWAY = 4                     # d_mm emitted this many fc ahead of combine

_SIM_SAFE_GELU = False         # CoreSim lacks Gelu; swap in Tanh for sim runs

_CACHE = {}


def _build_graph():
    fp32 = mybir.dt.float32
    bf16 = mybir.dt.bfloat16

    nc = bacc.Bacc("TRN2", target_bir_lowering=False, debug=False,
                   num_devices=NCORES)

    # ---- DRAM parameters (per-core shards; same shapes on every core).
    # Startup tensors are host-packed so each is ONE contiguous DMA.
    hP = nc.dram_tensor("hP", [128, NKC * TP], bf16, kind="ExternalInput")
    cP = nc.dram_tensor("cP", [128, 216], fp32, kind="ExternalInput")
    wrP = nc.dram_tensor("wrP", [128, NKC * K], bf16, kind="ExternalInput")
    validT = nc.dram_tensor("validT", [NOFF, T], fp32, kind="ExternalInput")
    b2o = nc.dram_tensor("b2o", [NOFF, D + 2], bf16, kind="ExternalInput")
    w1ab = nc.dram_tensor("w1ab", [NFC, 128, 2 * D], bf16,
                          kind="ExternalInput")
    w2 = nc.dram_tensor("w2", [NFC, 128, D], bf16, kind="ExternalInput")
    out = nc.dram_tensor("out", [T, D], fp32, kind="ExternalOutput")

    AF = mybir.ActivationFunctionType
    OP = mybir.AluOpType

    def bc_ap(tile_, inner_rep, ncols):
        """[128, ncols] tile viewed as [128, ncols, inner_rep] via a step-0
        innermost dim (per-partition broadcast along the replicated axis)."""
        return bass_rust.AP(
            tensor=tile_[:].tensor, offset=0,
            ap=[[ncols, 128], [1, ncols], [0, inner_rep]])

    def rows_ap(tile_, off, ostep, ocnt, icnt):
        """Multi-row free AP: ocnt rows of icnt step-1 elements, row starts
        off, off+ostep, ...  (all starts must be 4B-aligned for bf16 2x)."""
        return bass_rust.AP(
            tensor=tile_[:].tensor, offset=off,
            ap=[[tile_[:].shape[1], 128], [ostep, ocnt], [1, icnt]])

    with tile.TileContext(nc) as tc:
        with (
            tc.tile_pool(name="const", bufs=1) as cpool,
            tc.tile_pool(name="hpool", bufs=1) as hpool,
            tc.tile_pool(name="w2pool", bufs=1) as w2pool,
            tc.tile_pool(name="w1pool", bufs=4) as w1pool,
            tc.tile_pool(name="small", bufs=2) as spool,
            tc.tile_pool(name="persist", bufs=1) as ppool,
            tc.tile_pool(name="uv", bufs=4) as uvpool,
            tc.tile_pool(name="big", bufs=RUNWAY + 1) as bigpool,
            tc.tile_pool(name="qbuf", bufs=1) as q1pool,
            tc.tile_pool(name="tbuf", bufs=1) as qpool,
            tc.tile_pool(name="ppart", bufs=1) as partpool,
            tc.tile_pool(name="gout", bufs=1) as gpool,
            tc.tile_pool(name="opool", bufs=4) as opool,
            tc.tile_pool(name="dram", bufs=1, space="DRAM") as dpool,
            tc.tile_pool(name="ps_big", bufs=4, space="PSUM") as psb,
            tc.tile_pool(name="ps_vb", bufs=1, space="PSUM") as psvb,
            tc.tile_pool(name="ps_small", bufs=3, space="PSUM") as pss,
            # PSUM budget (8 banks): psb "m" 4 (u/va double buffer; the 4
            # banks are reused for delta preopens once stage D ends),
            # psvb "vb" 1, pss "s" 3 (logits, transposes, den, E1
            # transients, and the 3 held delta groups).
        ):
            # ---------------- packed startup loads ----------------
            h_m = hpool.tile([128, NKC * TP], bf16, tag="h")
            nc.sync.dma_start(h_m[:], hP[:])
            c_m = cpool.tile([128, 216], fp32, tag="c")
            nc.sync.dma_start(c_m[:], cP[:])
            wr_m = cpool.tile([128, NKC * K], bf16, tag="wr")
            nc.sync.dma_start(wr_m[:], wrP[:])
            validT_sb = cpool.tile([NOFF, T], fp32, tag="validT")
            nc.sync.dma_start(validT_sb[:], validT[:])
            b2o_sb = cpool.tile([NOFF, D + 2], bf16, tag="b2o")
            nc.sync.dma_start(b2o_sb[:], b2o[:])

            h_sb = [h_m[:, kc * TP:(kc + 1) * TP] for kc in range(NKC)]
            ident_sb = c_m[:, 0:128]
            br_sb = c_m[:, 128:160]
            b1_sb = c_m[:, 160:176]
            vtok_sb = c_m[:, 176:216]
            wr_sb = [wr_m[:, kc * K:(kc + 1) * K] for kc in range(NKC)]
            b2_sb = b2o_sb[0:K, 0:D]
            ones10_sb = b2o_sb[:, D:D + 1]

            # fc0's W1 rides the (idle) Tensor-engine DMA queue so it
            # lands in parallel with the sync queue's h/const loads.
            w1_first = w1pool.tile([128, 2 * D], bf16, tag="w1")
            nc.scalar.dma_start(w1_first[:], w1ab[0])

            # Hoist both ACT table loads (exp + gelu sets, ~1.3us each) into
            # the h-DMA shadow via 1-column dummy activations.
            warm = spool.tile([1, 1], fp32, tag="warm")
            nc.scalar.activation(warm[:], ident_sb[0:1, 0:1], AF.Exp)
            nc.scalar.activation(warm[:], warm[:], AF.Tanh if _SIM_SAFE_GELU else AF.Gelu)

            # persistent transposed score & router weights (bf16)
            cwT_bf = ppool.tile([NOFF, T], bf16, tag="cwT")
            wT_bf = ppool.tile([K, T], bf16, tag="wT")
            cw_bc = gpool.tile([128, NOFF * 512], bf16, tag="cw_bc")
            w_bc_all = gpool.tile([128, K * 512], bf16, tag="w_bc_all")

            # ------------- stage A/B/C: scores, cw, router w -------------
            # Phase A: gram/router matmuls; each tile's diagonal extraction
            # is emitted right behind its gram so DVE starts ASAP; the logit
            # evac follows immediately so only one "s" bank is held per tile.
            s_all = spool.tile([128, NTC * NOFF], fp32, tag="s_all")
            junk = spool.tile([128, 128], fp32, tag="junk")
            lg_all = spool.tile([128, NTC * K], fp32, tag="lg_all")

            def phase_gram(tci):
                c0 = PADL + tci * 128
                g_ps = psb.tile([128, 512], fp32, tag="m")
                lg_ps = pss.tile([128, K], fp32, tag="s")
                for kc in range(NKC):
                    st = (kc == 0)
                    sp = (kc == NKC - 1)
                    nc.tensor.matmul(g_ps[:, :138],
                                     h_sb[kc][:, c0:c0 + 128],
                                     h_sb[kc][:, c0 - 5:c0 + 133],
                                     start=st, stop=sp)
                    nc.tensor.matmul(lg_ps[:],
                                     h_sb[kc][:, c0:c0 + 128],
                                     wr_sb[kc],
                                     start=st, stop=sp)
                for n, off in enumerate(OFF_ORDER):
                    nc.vector.affine_mul_reduce(
                        junk[:], s_all[:, tci * NOFF + n:tci * NOFF + n + 1],
                        g_ps[:, off + 5:off + 5 + 128], ident_sb,
                        1.0 / 32.0, 0.0)
                nc.scalar.copy(lg_all[:, tci * K:(tci + 1) * K], lg_ps[:])

            # Phase B1: transpose scores to [NOFF, T], then broadcast the
            # UNNORMALIZED ev = exp(s)*valid right away.  The softmax 1/den
            # lands at the very end as a per-token (=per-partition) scale on
            # the delta close.  No max-shift is needed: scores are O(1) so
            # exp() cannot over/underflow.
            sT = ppool.tile([NOFF, T], fp32, tag="sT")

            def phase_b1():
                for tci in range(NTC):
                    sT_ps = pss.tile([NOFF, 128], fp32, tag="s")
                    nc.tensor.transpose(sT_ps[:],
                                        s_all[:, tci * NOFF:(tci + 1) * NOFF],
                                        ident_sb)
                    nc.scalar.copy(sT[:, tci * 128:(tci + 1) * 128], sT_ps[:])
                evT = ppool.tile([NOFF, T], fp32, tag="evT")
                nc.scalar.activation(evT[:], sT[:], AF.Exp)
                nc.vector.tensor_mul(cwT_bf[:], evT[:], validT_sb[:])
                cw_dram = dpool.tile([1, NOFF * T], bf16, tag="cw_dram")
                nc.scalar.dma_start(cw_dram[:], cwT_bf[:])
                nc.scalar.dma_start(cw_bc[:],
                                    cw_dram[:].partition_broadcast(128))

            rdenT = ppool.tile([128, NTC], fp32, tag="rdenT")
            fT = ppool.tile([128, NTC], fp32, tag="fT")
            wplT_bf = ppool.tile([K, T], bf16, tag="wplT")
            w_pl = ppool.tile([128, NTC * K], fp32, tag="w_pl")

            we = spool.tile([128, NTC * K], fp32, tag="we")

            def phase_b2a_pre():
                """Router softmax up to the exp."""
                nc.vector.tensor_add(lg_all[:], lg_all[:], br_sb)
                wmx = spool.tile([128, NTC], fp32, tag="wmx")
                lg3 = bass_rust.AP(tensor=lg_all[:].tensor, offset=0,
                                   ap=[[NTC * K, 128], [K, NTC], [1, K]])
                nc.vector.reduce_max(wmx[:], lg3, mybir.AxisListType.X)
                nc.vector.tensor_sub(we[:], lg_all[:], bc_ap(wmx, K, NTC))
                nc.scalar.activation(we[:], we[:], AF.Exp)

            def phase_b2a_post():
                """Post-exp half: w softmax, transpose, broadcast.  Emitted
                before the first gelu so the w_dram round trip isn't queued
                behind 5.5us ACT gelus."""
                wsum = spool.tile([128, NTC], fp32, tag="wsum")
                we3 = bass_rust.AP(tensor=we[:].tensor, offset=0,
                                   ap=[[NTC * K, 128], [K, NTC], [1, K]])
                nc.vector.reduce_sum(wsum[:], we3, mybir.AxisListType.X)
                rws = spool.tile([128, NTC], fp32, tag="rws")
                nc.vector.reciprocal(rws[:], wsum[:])
                nc.vector.tensor_mul(w_pl[:], we[:], bc_ap(rws, K, NTC))
                for tci in range(NTC):
                    wpT_ps = pss.tile([K, 128], fp32, tag="s")
                    nc.tensor.transpose(wpT_ps[:],
                                        w_pl[:, tci * K:(tci + 1) * K],
                                        ident_sb)
                    nc.scalar.copy(wplT_bf[:, tci * 128:(tci + 1) * 128],
                                   wpT_ps[:])
                w_dram = dpool.tile([1, K * T], bf16, tag="w_dram")
                nc.scalar.dma_start(w_dram[:], wplT_bf[:])
                nc.scalar.dma_start(w_bc_all[:],
                                    w_dram[:].partition_broadcast(128))

            weff3 = spool.tile([128, NTC * K], fp32, tag="weff3")

            def phase_b2b_early():
                """Denominator per token, computed token-major entirely on
                DVE (+1 tiny ACT exp) so no PE matmul or DMA round trip sits
                on the early critical path.  The reciprocal runs on the
                [128, NTC] layout -- a [1, T] reciprocal would serialize 512
                8-cycle divides on one partition (~4us)."""
                evm = spool.tile([128, NTC * NOFF], fp32, tag="evm")
                nc.scalar.activation(evm[:], s_all[:], AF.Exp)
                nc.vector.tensor_mul(evm[:], evm[:], vtok_sb)
                ev3 = bass_rust.AP(tensor=evm[:].tensor, offset=0,
                                   ap=[[NTC * NOFF, 128], [NOFF, NTC],
                                       [1, NOFF]])
                nc.vector.reduce_sum(fT[:], ev3, mybir.AxisListType.X)
                dene_t = ppool.tile([128, NTC], fp32, tag="dene_t")
                nc.vector.tensor_scalar_add(dene_t[:], fT[:], 1e-30)
                nc.vector.reciprocal(rdenT[:], dene_t[:])
                # b2-path weights: w * raw_den (so the final 1/den scale on
                # the delta close reproduces w * sum_cw exactly)
                for tci in range(NTC):
                    nc.vector.tensor_scalar_mul(
                        weff3[:, tci * K:(tci + 1) * K],
                        w_pl[:, tci * K:(tci + 1) * K], fT[:, tci:tci + 1])

            def phase_b2b_late():
                """Tiny weff transposes; deferred so they never head-of-line
                block the PE queue while waiting on weff3."""
                for tci in range(NTC):
                    weT_ps = pss.tile([K, 128], fp32, tag="s")
                    nc.tensor.transpose(weT_ps[:],
                                        weff3[:, tci * K:(tci + 1) * K],
                                        ident_sb)
                    nc.scalar.copy(wT_bf[:, tci * 128:(tci + 1) * 128],
                                   weT_ps[:])

            # ------------- stage D: u/v matmuls + gelu combine -------------
            g_sb = [None] * NFC
            tmp_sb = [None] * NFC

            def stage_d_mm(fc, w1_pre=None):
                if w1_pre is None:
                    w1_t = w1pool.tile([128, 2 * D], bf16, tag="w1")
                    nc.sync.dma_start(w1_t[:], w1ab[fc])
                else:
                    w1_t = w1_pre

                u_ps = psb.tile([128, 512], fp32, tag="m")
                va_ps = psb.tile([128, 512], fp32, tag="m")
                vb_ps = psvb.tile([128, 48], fp32, tag="vb")
                for kc in range(NKC):
                    st = (kc == 0)
                    sp = (kc == NKC - 1)
                    lhs_b = w1_t[:, kc * 128:(kc + 1) * 128]
                    lhs_a = w1_t[:, D + kc * 128:D + (kc + 1) * 128]
                    nc.tensor.matmul(u_ps[:], lhs_b,
                                     h_sb[kc][:, PADL:PADL + 512],
                                     start=st, stop=sp)
                    nc.tensor.matmul(va_ps[:], lhs_a,
                                     h_sb[kc][:, 0:512],
                                     start=st, stop=sp)
                    nc.tensor.matmul(vb_ps[:], lhs_a,
                                     h_sb[kc][:, 496:544],
                                     start=st, stop=sp)

                u_sb = uvpool.tile([128, 512], bf16, tag="u")
                nc.scalar.copy(u_sb[:], u_ps[:])
                v_ev = uvpool.tile([128, TP], bf16, tag="v_ev")
                nc.scalar.copy(v_ev[:, 0:512], va_ps[:])
                nc.scalar.copy(v_ev[:, 512:544], vb_ps[:, 16:48])
                # odd phase built straight from PSUM (keeps DMA out of the
                # critical chain)
                v_od = uvpool.tile([128, TP], bf16, tag="v_od")
                nc.scalar.copy(v_od[:, 0:511], va_ps[:, 1:512])
                nc.scalar.copy(v_od[:, 511:543], vb_ps[:, 16:48])

                tmp = bigpool.tile([128, NOFF * 512], bf16, tag="tmp")
                # Batched shifted adds: every row start is an even element
                # index (4B-aligned), so the multi-row APs keep the DVE's
                # bf16 2x packing (hardware-verified: 6-row 1752ns vs
                # 6x418ns single-row).  Layout matches OFF_ORDER:
                #   [0:1024)    offs -4,-2    from v_ev
                #   [1024:2048) offs 2,4      from v_ev
                #   [2048:5120) offs -5..5 odd from v_od
                nc.vector.tensor_add(
                    rows_ap(tmp, 0, 512, 2, 512),
                    rows_ap(v_ev, PADL - 4, 2, 2, 512),
                    rows_ap(u_sb, 0, 0, 2, 512))
                nc.vector.tensor_add(
                    rows_ap(tmp, 1024, 512, 2, 512),
                    rows_ap(v_ev, PADL + 2, 2, 2, 512),
                    rows_ap(u_sb, 0, 0, 2, 512))
                nc.vector.tensor_add(
                    rows_ap(tmp, 2048, 512, 6, 512),
                    rows_ap(v_od, PADL - 1 - 5, 2, 6, 512),
                    rows_ap(u_sb, 0, 0, 6, 512))
                nc.scalar.activation(tmp[:], tmp[:], AF.Tanh if _SIM_SAFE_GELU else AF.Gelu,
                                     bias=b1_sb[:, fc:fc + 1])
                tmp_sb[fc] = tmp

            def stage_d_combine(fc):
                tmp = tmp_sb[fc]
                q = q1pool.tile([128, NOFF * 512], bf16, tag="q")
                nc.vector.tensor_mul(q[:], tmp[:], cw_bc[:])

                # pairwise tree-sum of the 10 weighted slices, then w-scale
                t1 = qpool.tile([128, 2560], bf16, tag="t1")
                nc.vector.tensor_add(t1[:], q[:, 0:2560], q[:, 2560:5120])
                t2 = qpool.tile([128, 1024], bf16, tag="t2")
                nc.vector.tensor_add(t2[:], t1[:, 0:1024], t1[:, 1024:2048])
                t3 = qpool.tile([128, 512], bf16, tag="t3")
                nc.vector.tensor_add(t3[:], t2[:, 0:512], t2[:, 512:1024])
                t4 = qpool.tile([128, 512], bf16, tag="t4")
                nc.vector.tensor_add(t4[:], t3[:], t1[:, 2048:2560])
                g_t = gpool.tile([128, 512], bf16, tag=f"g{fc}")
                nc.vector.tensor_mul(
                    g_t[:], t4[:],
                    w_bc_all[:, (fc // 2) * 512:(fc // 2) * 512 + 512])
                g_sb[fc] = g_t

            w2_sb = [None] * NFC

            def load_w2(j):
                t = w2pool.tile([128, D], bf16, tag=f"w2_{j}")
                nc.sync.dma_start(t[:], w2[j])
                w2_sb[j] = t

            def blk_mm(d_ps, blk, fc, start, stop=False):
                tci, dh = blk // 2, blk % 2
                nc.tensor.matmul(
                    d_ps[:],
                    g_sb[fc][:, tci * 128:(tci + 1) * 128],
                    w2_sb[fc][:, dh * 512:(dh + 1) * 512],
                    start=start, stop=stop)

            def blk_b2_mm(d_ps, blk):
                tci, dh = blk // 2, blk % 2
                nc.tensor.matmul(
                    d_ps[:],
                    wT_bf[:, tci * 128:(tci + 1) * 128],
                    b2_sb[:, dh * 512:(dh + 1) * 512],
                    start=False, stop=True)

            def out_dma(o_sb, blk):
                tci, dh = blk // 2, blk % 2
                nc.sync.dma_start(
                    out[tci * 128:(tci + 1) * 128,
                        dh * 512:(dh + 1) * 512], o_sb[:])

            # -- delta groups.  Held groups (blocks 0..2 on "s" banks, and
            # preopened blocks 3..6 on freed "m" banks) accumulate fc matmuls
            # per combine and close with b2 + a per-token 1/den scale.
            open_ps = {}

            def grp_open(blk, g_lo, g_hi, pool):
                d_ps = pool.tile([128, 512],
                                 mybir.dt.float32, tag="m" if pool is psb
                                 else "s", name=f"dps{blk}")
                for fc in range(g_lo, g_hi + 1):
                    blk_mm(d_ps, blk, fc, start=(fc == g_lo))
                open_ps[blk] = d_ps

            def grp_extend(blk, fc):
                blk_mm(open_ps[blk], blk, fc, start=False)

            def grp_close_direct(blk):
                """For groups that accumulated all of fc 0..15."""
                tci = blk // 2
                d_ps = open_ps[blk]
                blk_b2_mm(d_ps, blk)
                o_sb = opool.tile([128, 512], fp32, tag="o")
                nc.scalar.mul(o_sb[:], d_ps[:], rdenT[:, tci:tci + 1])
                out_dma(o_sb, blk)

            d_part = {}

            def stage_e1(blk):
                """fc 0..7 partial for blocks 3..7 (one pss bank transient)."""
                tci = blk // 2
                d_ps = pss.tile([128, 512], fp32, tag="s")
                for fc in range(8):
                    blk_mm(d_ps, blk, fc, start=(fc == 0), stop=(fc == 7))
                p_t = partpool.tile([128, 512], bf16, tag=f"p{blk}")
                nc.scalar.mul(p_t[:], d_ps[:], rdenT[:, tci:tci + 1])
                d_part[blk] = p_t

            def grp_close_merge(blk):
                """For groups that accumulated fc 8..15: merge with the E1
                partial via one scalar_tensor_tensor."""
                tci = blk // 2
                d_ps = open_ps[blk]
                blk_b2_mm(d_ps, blk)
                o_sb = opool.tile([128, 512], fp32, tag="o")
                nc.vector.scalar_tensor_tensor(
                    o_sb[:], d_ps[:], rdenT[:, tci:tci + 1], d_part[blk][:],
                    op0=OP.mult, op1=OP.add)
                out_dma(o_sb, blk)

            # ---- emission schedule ----
            phase_gram(0)
            phase_gram(1)
            stage_d_mm(0, w1_pre=w1_first)
            phase_gram(2)
            phase_gram(3)
            phase_b1()
            phase_b2a_pre()
            phase_b2a_post()
            phase_b2b_early()
            for fc in range(1, RUNWAY):
                stage_d_mm(fc)
            for j in range(NFC):            # combine index
                jj = j + RUNWAY
                if jj < NFC:
                    stage_d_mm(jj)
                    if 4 <= jj <= 11:
                        load_w2(2 * (jj - 4))
                        load_w2(2 * (jj - 4) + 1)
                stage_d_combine(j)
                if j == 1:
                    phase_b2b_late()
                if 7 <= j <= 10:
                    stage_e1(j - 4)         # blocks 3..6
                if j == 11:
                    stage_e1(7)
                    grp_open(0, 0, 11, pss)         # held, g0..11
                if j == 12:
                    grp_extend(0, 12)
                    grp_open(1, 0, 12, pss)         # held, g0..12
                    for blk in range(3, 7):         # preopens on freed m
                        grp_open(blk, 8, 12, psb)
                if j == 13:
                    for blk in (0, 1, 3, 4, 5, 6):
                        grp_extend(blk, 13)
                    grp_open(2, 0, 13, pss)         # held, g0..13
                if j == 14:
                    for blk in (0, 1, 2, 3, 4, 5, 6):
                        grp_extend(blk, 14)
            # tail: one g15 + b2 per open group, then block 7 full
            for blk in (0, 1, 2, 3, 4, 5, 6):
                grp_extend(blk, 15)
            grp_close_direct(0)
            grp_open(7, 8, 15, pss)
            grp_close_direct(1)
            grp_close_direct(2)
            for blk in (3, 4, 5, 6):
                grp_close_merge(blk)
            grp_close_merge(7)

    nc.compile()
    return nc


def _prep_shards(h_L, mask_flags, Wr, br, W1, b1, W2, b2):
    """Host-side shard construction (numpy only; cheap vs device work)."""
    f32 = np.float32
    h_L = np.asarray(h_L, f32)
    mask = np.asarray(mask_flags)
    Wr = np.asarray(Wr, f32)
    W1 = np.asarray(W1, f32)
    W2 = np.asarray(W2, f32)
    br = np.asarray(br, f32)
    b1 = np.asarray(b1, f32)
    b2 = np.asarray(b2, f32)

    # shared (replicated) weight blocks
    w1a = np.ascontiguousarray(
        W1[:, :D, :].transpose(1, 0, 2).reshape(D, F)
        .reshape(NKC, 128, NFC, 128).transpose(2, 1, 0, 3)
        .reshape(NFC, 128, D)).astype(BF16)
    w1b = np.ascontiguousarray(
        W1[:, D:, :].transpose(1, 0, 2).reshape(D, F)
        .reshape(NKC, 128, NFC, 128).transpose(2, 1, 0, 3)
        .reshape(NFC, 128, D)).astype(BF16)
    w1ab = np.concatenate([w1b, w1a], axis=2)        # [NFC, 128, 2D]
    w2p = np.ascontiguousarray(
        W2.reshape(F, D).reshape(NFC, 128, D)).astype(BF16)
    # packed wr: [128, NKC*K]
    wrP = np.ascontiguousarray(
        Wr.reshape(NKC, 128, K).transpose(1, 0, 2).reshape(128, NKC * K)
    ).astype(BF16)
    # packed consts: ident | br_bc | b1s  -> [128, 176] fp32
    br_bc = np.tile(np.broadcast_to(br[None, :], (128, K)), (1, NTC)).astype(f32)
    b1s = np.ascontiguousarray(b1.reshape(F).reshape(NFC, 128).T)
    cP = np.concatenate([np.eye(128, dtype=f32), br_bc, b1s], axis=1)
    # packed b2 + ones column: [NOFF, D+1] bf16
    b2o = np.zeros((NOFF, D + 2), BF16)
    b2o[:K, :D] = b2.astype(BF16)
    b2o[:, D] = 1.0

    offs = np.array(OFF_ORDER, np.int64)
    in_maps = []
    outs_meta = []
    per_batch = L // (NCORES // B)          # 512 tokens, 4 shards per batch
    for c in range(NCORES):
        b = c // (NCORES // B)
        t0 = (c % (NCORES // B)) * per_batch
        # padded, transposed h slice  [D, TP] -> packed [128, NKC*TP]
        hpad = np.zeros((TP, D), f32)
        lo = t0 - PADL
        hi = t0 + T + PADL
        slo, shi = max(lo, 0), min(hi, L)
        hpad[slo - lo:shi - lo] = h_L[b, slo:shi]
        hTa = np.ascontiguousarray(hpad.T).astype(BF16)          # [D, TP]
        hP = np.ascontiguousarray(
            hTa.reshape(NKC, 128, TP).transpose(1, 0, 2)
            .reshape(128, NKC * TP))

        # validity per (token, offset-order) -> [NOFF, T]
        tok = t0 + np.arange(T)
        nbr = tok[:, None] + offs[None, :]
        inb = (nbr >= 0) & (nbr < L)
        nbrc = np.clip(nbr, 0, L - 1)
        is_m = (mask[b] == 1)
        val = (inb & is_m[tok][:, None] & (~is_m[nbrc])).astype(f32)
        valT = np.ascontiguousarray(val.T)            # [NOFF, T]
        vtok = np.ascontiguousarray(
            val.reshape(NTC, 128, NOFF).transpose(1, 0, 2)
            .reshape(128, NTC * NOFF))
        cPc = np.concatenate([cP, vtok], axis=1)
        in_maps.append({
            "hP": hP, "cP": cPc, "wrP": wrP, "validT": valT, "b2o": b2o,
            "w1ab": w1ab, "w2": w2p,
        })
        outs_meta.append((b, t0))
    return in_maps, outs_meta


def kernel(**inputs):
    assert int(inputs["range_r"]) == R
    if "nc" not in _CACHE:
        _CACHE["nc"] = _build_graph()
    nc = _CACHE["nc"]
    in_maps, outs_meta = _prep_shards(
        inputs["h_L"], inputs["mask_flags"], inputs["Wr"], inputs["br"],
        inputs["W1"], inputs["b1"], inputs["W2"], inputs["b2"])
    res = run_bass_kernel_spmd(nc, in_maps, core_ids=list(range(NCORES)))
    out = np.zeros((B, L, D), np.float32)
    for c, (b, t0) in enumerate(outs_meta):
        out[b, t0:t0 + T] = res.results[c]["out"]
    return out


# revision 14
# speedup vs baseline: 1.0117x; 1.0117x over previous
"""Trainium2 Bass kernel for nn_AMIPRouterInference (gnn_message_passing).

Strategy
--------
Algebraic restructure of the reference (~515 GFLOP -> ~52 GFLOP):
  * cond @ W1 splits into h_anc @ W1a + h_ctr @ W1b, each computed once per
    token (not once per window pair):  u = h @ W1b, v = h @ W1a.
  * The attention combine over the +-r window commutes with the W2 matmul:
    g = sum_n cw_n * gelu(v[l+off_n] + u[l]);  delta = (w * g) @ W2 + w @ b2.

Sharding: pure data-parallel over the B*L = 4096 tokens -> 512 tokens/core on
8 cores; the +-5 halo is baked into each core's input shard on the host, so no
collectives are needed.

Per-core layout: features-on-partitions (u/v as 16 chunks of [128, tokens]) so
window shifts along tokens are free-axis SBUF slices.  Even/odd phase copies of
v keep the bf16 DVE 2x alignment for shifted adds.

Key engine facts this schedule is built around:
  * DVE is the bottleneck engine (~165us of tensor_tensor at bf16 2x).
    Batched multi-row-AP adds keep the 2x packing when every row start is
    4B-aligned (hardware-verified).
  * PE clock is HAM-gated: 1.2 GHz cold, 2.4 GHz after ~3.4us of sustained
    activity; any >3.4us idle window re-throttles.  The delta-stage matmuls
    are paced per-combine through the back half so the post-combine(15)
    tail is only ~23 matmuls.
  * DMA issue costs ~650ns per descriptor on the in-order sync queue, so
    startup inputs are packed host-side into 5 large contiguous transfers.
  * A 4-fc emission runway (d_mm 0..3 before combine 0) gives the DVE queue
    adds-work to chew while the exp->broadcast round trip for cw lands;
    combines then trail d_mm by 4 fc for the rest of the kernel, which also
    keeps ACT's gelu well ahead of the combine that consumes it.
"""

import sys

for _p in ("/opt/trn_rl_repo", "/root/.axon_site/_ro/trn_rl_repo"):
    if _p not in sys.path:
        sys.path.append(_p)

import numpy as np
import ml_dtypes

import bass_rust
import concourse.bacc as bacc
import concourse.mybir as mybir
import concourse.tile as tile
from concourse.bass_utils import run_bass_kernel_spmd

BF16 = ml_dtypes.bfloat16

# Problem constants (hardcoded per spec).
B, L, D = 2, 2048, 1024
K, D4, R = 8, 256, 5
NCORES = 8
T = (B * L) // NCORES          # tokens per core = 512
PADL = 16                      # left pad of the per-core token window
TP = T + 2 * PADL              # padded width = 544
NOFF = 2 * R                   # 10 window offsets
F = K * D4                     # 2048 fused expert features
NFC = F // 128                 # 16 feature chunks
NKC = D // 128                 # 8 contraction chunks
NTC = T // 128                 # 4 token tiles per core

# Offset processing order: even offsets first (read from v_even), then odd
# (read from v_odd, which holds v shifted left by one token).  Within each
# phase every slice start is an even element index -> 4-byte aligned, which
# keeps the DVE's bf16 2x packing for the batched multi-row adds.
OFF_ORDER = [-4, -2, 2, 4, -5, -3, -1, 1, 3, 5]

RUNWAY = 4                     # d_mm emitted this many fc ahead of combine

_SIM_SAFE_GELU = False         # CoreSim lacks Gelu; swap in Tanh for sim runs

_CACHE = {}


def _build_graph():
    fp32 = mybir.dt.float32
    bf16 = mybir.dt.bfloat16

    nc = bacc.Bacc("TRN2", target_bir_lowering=False, debug=False,
                   num_devices=NCORES)

    # ---- DRAM parameters (per-core shards; same shapes on every core).
    # Startup tensors are host-packed so each is ONE contiguous DMA.
    hP = nc.dram_tensor("hP", [128, NKC * TP], bf16, kind="ExternalInput")
    cP = nc.dram_tensor("cP", [128, 216], fp32, kind="ExternalInput")
    wrP = nc.dram_tensor("wrP", [128, NKC * K], bf16, kind="ExternalInput")
    validT = nc.dram_tensor("validT", [NOFF, T], fp32, kind="ExternalInput")
    b2o = nc.dram_tensor("b2o", [NOFF, D + 2], bf16, kind="ExternalInput")
    w1ab = nc.dram_tensor("w1ab", [NFC, 128, 2 * D], bf16,
                          kind="ExternalInput")
    w2 = nc.dram_tensor("w2", [NFC, 128, D], bf16, kind="ExternalInput")
    out = nc.dram_tensor("out", [T, D], fp32, kind="ExternalOutput")

    AF = mybir.ActivationFunctionType
    OP = mybir.AluOpType

    def bc_ap(tile_, inner_rep, ncols):
        """[128, ncols] tile viewed as [128, ncols, inner_rep] via a step-0
        innermost dim (per-partition broadcast along the replicated axis)."""
        return bass_rust.AP(
            tensor=tile_[:].tensor, offset=0,
            ap=[[ncols, 128], [1, ncols], [0, inner_rep]])

    def rows_ap(tile_, off, ostep, ocnt, icnt):
        """Multi-row free AP: ocnt rows of icnt step-1 elements, row starts
        off, off+ostep, ...  (all starts must be 4B-aligned for bf16 2x)."""
        return bass_rust.AP(
            tensor=tile_[:].tensor, offset=off,
            ap=[[tile_[:].shape[1], 128], [ostep, ocnt], [1, icnt]])

    with tile.TileContext(nc) as tc:
        with (
            tc.tile_pool(name="const", bufs=1) as cpool,
            tc.tile_pool(name="hpool", bufs=1) as hpool,
            tc.tile_pool(name="w2pool", bufs=1) as w2pool,
            tc.tile_pool(name="w1pool", bufs=4) as w1pool,
            tc.tile_pool(name="small", bufs=2) as spool,
            tc.tile_pool(name="persist", bufs=1) as ppool,
            tc.tile_pool(name="uv", bufs=4) as uvpool,
            tc.tile_pool(name="big", bufs=RUNWAY + 1) as bigpool,
            tc.tile_pool(name="qbuf", bufs=1) as q1pool,
            tc.tile_pool(name="tbuf", bufs=1) as qpool,
            tc.tile_pool(name="ppart", bufs=1) as partpool,
            tc.tile_pool(name="gout", bufs=1) as gpool,
            tc.tile_pool(name="opool", bufs=4) as opool,
            tc.tile_pool(name="dram", bufs=1, space="DRAM") as dpool,
            tc.tile_pool(name="ps_big", bufs=4, space="PSUM") as psb,
            tc.tile_pool(name="ps_vb", bufs=1, space="PSUM") as psvb,
            tc.tile_pool(name="ps_small", bufs=3, space="PSUM") as pss,
            # PSUM budget (8 banks): psb "m" 4 (u/va double buffer; the 4
            # banks are reused for delta preopens once stage D ends),
            # psvb "vb" 1, pss "s" 3 (logits, transposes, den, E1
            # transients, and the 3 held delta groups).
        ):
            # ---------------- packed startup loads ----------------
            h_m = hpool.tile([128, NKC * TP], bf16, tag="h")
            nc.sync.dma_start(h_m[:], hP[:])
            c_m = cpool.tile([128, 216], fp32, tag="c")
            nc.sync.dma_start(c_m[:], cP[:])
            wr_m = cpool.tile([128, NKC * K], bf16, tag="wr")
            nc.sync.dma_start(wr_m[:], wrP[:])
            validT_sb = cpool.tile([NOFF, T], fp32, tag="validT")
            nc.sync.dma_start(validT_sb[:], validT[:])
            b2o_sb = cpool.tile([NOFF, D + 2], bf16, tag="b2o")
            nc.sync.dma_start(b2o_sb[:], b2o[:])

            h_sb = [h_m[:, kc * TP:(kc + 1) * TP] for kc in range(NKC)]
            ident_sb = c_m[:, 0:128]
            br_sb = c_m[:, 128:160]
            b1_sb = c_m[:, 160:176]
            vtok_sb = c_m[:, 176:216]
            wr_sb = [wr_m[:, kc * K:(kc + 1) * K] for kc in range(NKC)]
            b2_sb = b2o_sb[0:K, 0:D]
            ones10_sb = b2o_sb[:, D:D + 1]

            # fc0's W1 rides the (idle) Tensor-engine DMA queue so it
            # lands in parallel with the sync queue's h/const loads.
            w1_first = w1pool.tile([128, 2 * D], bf16, tag="w1")
            nc.scalar.dma_start(w1_first[:], w1ab[0])

            # Hoist both ACT table loads (exp + gelu sets, ~1.3us each) into
            # the h-DMA shadow via 1-column dummy activations.
            warm = spool.tile([1, 1], fp32, tag="warm")
            nc.scalar.activation(warm[:], ident_sb[0:1, 0:1], AF.Exp)

            # persistent transposed score & router weights (bf16)
            cwT_bf = ppool.tile([NOFF, T], bf16, tag="cwT")
            wT_bf = ppool.tile([K, T], bf16, tag="wT")
            cw_bc = gpool.tile([128, NOFF * 512], bf16, tag="cw_bc")
            w_bc_all = gpool.tile([128, K * 512], bf16, tag="w_bc_all")

            # ------------- stage A/B/C: scores, cw, router w -------------
            # Phase A: gram/router matmuls; each tile's diagonal extraction
            # is emitted right behind its gram so DVE starts ASAP; the logit
            # evac follows immediately so only one "s" bank is held per tile.
            s_all = spool.tile([128, NTC * NOFF], fp32, tag="s_all")
            junk = spool.tile([128, 128], fp32, tag="junk")
            lg_all = spool.tile([128, NTC * K], fp32, tag="lg_all")

            def phase_gram(tci):
                c0 = PADL + tci * 128
                g_ps = psb.tile([128, 512], fp32, tag="m")
                lg_ps = pss.tile([128, K], fp32, tag="s")
                for kc in range(NKC):
                    st = (kc == 0)
                    sp = (kc == NKC - 1)
                    nc.tensor.matmul(g_ps[:, :138],
                                     h_sb[kc][:, c0:c0 + 128],
                                     h_sb[kc][:, c0 - 5:c0 + 133],
                                     start=st, stop=sp)
                    nc.tensor.matmul(lg_ps[:],
                                     h_sb[kc][:, c0:c0 + 128],
                                     wr_sb[kc],
                                     start=st, stop=sp)
                for n, off in enumerate(OFF_ORDER):
                    nc.vector.affine_mul_reduce(
                        junk[:], s_all[:, tci * NOFF + n:tci * NOFF + n + 1],
                        g_ps[:, off + 5:off + 5 + 128], ident_sb,
                        1.0 / 32.0, 0.0)
                nc.scalar.copy(lg_all[:, tci * K:(tci + 1) * K], lg_ps[:])

            # Phase B1: transpose scores to [NOFF, T], then broadcast the
            # UNNORMALIZED ev = exp(s)*valid right away.  The softmax 1/den
            # lands at the very end as a per-token (=per-partition) scale on
            # the delta close.  No max-shift is needed: scores are O(1) so
            # exp() cannot over/underflow.
            sT = ppool.tile([NOFF, T], fp32, tag="sT")

            def phase_b1():
                for tci in range(NTC):
                    sT_ps = pss.tile([NOFF, 128], fp32, tag="s")
                    nc.tensor.transpose(sT_ps[:],
                                        s_all[:, tci * NOFF:(tci + 1) * NOFF],
                                        ident_sb)
                    nc.scalar.copy(sT[:, tci * 128:(tci + 1) * 128], sT_ps[:])
                evT = ppool.tile([NOFF, T], fp32, tag="evT")
                nc.scalar.activation(evT[:], sT[:], AF.Exp)
                nc.vector.tensor_mul(cwT_bf[:], evT[:], validT_sb[:])
                cw_dram = dpool.tile([1, NOFF * T], bf16, tag="cw_dram")
                nc.scalar.dma_start(cw_dram[:], cwT_bf[:])
                nc.scalar.dma_start(cw_bc[:],
                                    cw_dram[:].partition_broadcast(128))

            rdenT = ppool.tile([128, NTC], fp32, tag="rdenT")
            fT = ppool.tile([128, NTC], fp32, tag="fT")
            wplT_bf = ppool.tile([K, T], bf16, tag="wplT")
            w_pl = ppool.tile([128, NTC * K], fp32, tag="w_pl")

            we = spool.tile([128, NTC * K], fp32, tag="we")

            def phase_b2a_pre():
                """Router softmax up to the exp."""
                nc.vector.tensor_add(lg_all[:], lg_all[:], br_sb)
                wmx = spool.tile([128, NTC], fp32, tag="wmx")
                lg3 = bass_rust.AP(tensor=lg_all[:].tensor, offset=0,
                                   ap=[[NTC * K, 128], [K, NTC], [1, K]])
                nc.vector.reduce_max(wmx[:], lg3, mybir.AxisListType.X)
                nc.vector.tensor_sub(we[:], lg_all[:], bc_ap(wmx, K, NTC))
                nc.scalar.activation(we[:], we[:], AF.Exp)

            def phase_b2a_post():
                """Post-exp half: w softmax, transpose, broadcast.  Emitted
                before the first gelu so the w_dram round trip isn't queued
                behind 5.5us ACT gelus."""
                wsum = spool.tile([128, NTC], fp32, tag="wsum")
                we3 = bass_rust.AP(tensor=we[:].tensor, offset=0,
                                   ap=[[NTC * K, 128], [K, NTC], [1, K]])
                nc.vector.reduce_sum(wsum[:], we3, mybir.AxisListType.X)
                rws = spool.tile([128, NTC], fp32, tag="rws")
                nc.vector.reciprocal(rws[:], wsum[:])
                nc.vector.tensor_mul(w_pl[:], we[:], bc_ap(rws, K, NTC))
                for tci in range(NTC):
                    wpT_ps = pss.tile([K, 128], fp32, tag="s")
                    nc.tensor.transpose(wpT_ps[:],
                                        w_pl[:, tci * K:(tci + 1) * K],
                                        ident_sb)
                    nc.scalar.copy(wplT_bf[:, tci * 128:(tci + 1) * 128],
                                   wpT_ps[:])
                w_dram = dpool.tile([1, K * T], bf16, tag="w_dram")
                nc.scalar.dma_start(w_dram[:], wplT_bf[:])
                nc.scalar.dma_start(w_bc_all[:],
                                    w_dram[:].partition_broadcast(128))

            weff3 = spool.tile([128, NTC * K], fp32, tag="weff3")

            def phase_b2b_early():
                """Denominator per token, computed token-major entirely on
                DVE (+1 tiny ACT exp) so no PE matmul or DMA round trip sits
                on the early critical path.  The reciprocal runs on the
                [128, NTC] layout -- a [1, T] reciprocal would serialize 512
                8-cycle divides on one partition (~4us)."""
                evm = spool.tile([128, NTC * NOFF], fp32, tag="evm")
                nc.scalar.activation(evm[:], s_all[:], AF.Exp)
                nc.vector.tensor_mul(evm[:], evm[:], vtok_sb)
                ev3 = bass_rust.AP(tensor=evm[:].tensor, offset=0,
                                   ap=[[NTC * NOFF, 128], [NOFF, NTC],
                                       [1, NOFF]])
                nc.vector.reduce_sum(fT[:], ev3, mybir.AxisListType.X)
                dene_t = ppool.tile([128, NTC], fp32, tag="dene_t")
                nc.vector.tensor_scalar_add(dene_t[:], fT[:], 1e-30)
                nc.vector.reciprocal(rdenT[:], dene_t[:])
                # b2-path weights: w * raw_den (so the final 1/den scale on
                # the delta close reproduces w * sum_cw exactly)
                for tci in range(NTC):
                    nc.vector.tensor_scalar_mul(
                        weff3[:, tci * K:(tci + 1) * K],
                        w_pl[:, tci * K:(tci + 1) * K], fT[:, tci:tci + 1])

            def phase_b2b_late():
                """Tiny weff transposes; deferred so they never head-of-line
                block the PE queue while waiting on weff3."""
                for tci in range(NTC):
                    weT_ps = pss.tile([K, 128], fp32, tag="s")
                    nc.tensor.transpose(weT_ps[:],
                                        weff3[:, tci * K:(tci + 1) * K],
                                        ident_sb)
                    nc.scalar.copy(wT_bf[:, tci * 128:(tci + 1) * 128],
                                   weT_ps[:])

            # ------------- stage D: u/v matmuls + gelu combine -------------
            g_sb = [None] * NFC
            tmp_sb = [None] * NFC

            def stage_d_gelu(fc):
                tmp = tmp_sb[fc]
                nc.scalar.activation(tmp[:], tmp[:],
                                     AF.Tanh if _SIM_SAFE_GELU else AF.Gelu,
                                     bias=b1_sb[:, fc:fc + 1])

            def stage_d_mm(fc, w1_pre=None, emit_gelu=True):
                if w1_pre is None:
                    w1_t = w1pool.tile([128, 2 * D], bf16, tag="w1")
                    nc.sync.dma_start(w1_t[:], w1ab[fc])
                else:
                    w1_t = w1_pre

                u_ps = psb.tile([128, 512], fp32, tag="m")
                va_ps = psb.tile([128, 512], fp32, tag="m")
                vb_ps = psvb.tile([128, 48], fp32, tag="vb")
                for kc in range(NKC):
                    st = (kc == 0)
                    sp = (kc == NKC - 1)
                    lhs_b = w1_t[:, kc * 128:(kc + 1) * 128]
                    lhs_a = w1_t[:, D + kc * 128:D + (kc + 1) * 128]
                    nc.tensor.matmul(u_ps[:], lhs_b,
                                     h_sb[kc][:, PADL:PADL + 512],
                                     start=st, stop=sp)
                    nc.tensor.matmul(va_ps[:], lhs_a,
                                     h_sb[kc][:, 0:512],
                                     start=st, stop=sp)
                    nc.tensor.matmul(vb_ps[:], lhs_a,
                                     h_sb[kc][:, 496:544],
                                     start=st, stop=sp)

                u_sb = uvpool.tile([128, 512], bf16, tag="u")
                nc.scalar.copy(u_sb[:], u_ps[:])
                v_ev = uvpool.tile([128, TP], bf16, tag="v_ev")
                nc.scalar.copy(v_ev[:, 0:512], va_ps[:])
                nc.scalar.copy(v_ev[:, 512:544], vb_ps[:, 16:48])
                # odd phase built straight from PSUM (keeps DMA out of the
                # critical chain)
                v_od = uvpool.tile([128, TP], bf16, tag="v_od")
                nc.scalar.copy(v_od[:, 0:511], va_ps[:, 1:512])
                nc.scalar.copy(v_od[:, 511:543], vb_ps[:, 16:48])

                tmp = bigpool.tile([128, NOFF * 512], bf16, tag="tmp")
                # Batched shifted adds: every row start is an even element
                # index (4B-aligned), so the multi-row APs keep the DVE's
                # bf16 2x packing (hardware-verified: 6-row 1752ns vs
                # 6x418ns single-row).  Layout matches OFF_ORDER:
                #   [0:1024)    offs -4,-2    from v_ev
                #   [1024:2048) offs 2,4      from v_ev
                #   [2048:5120) offs -5..5 odd from v_od
                nc.vector.tensor_add(
                    rows_ap(tmp, 0, 512, 2, 512),
                    rows_ap(v_ev, PADL - 4, 2, 2, 512),
                    rows_ap(u_sb, 0, 0, 2, 512))
                nc.vector.tensor_add(
                    rows_ap(tmp, 1024, 512, 2, 512),
                    rows_ap(v_ev, PADL + 2, 2, 2, 512),
                    rows_ap(u_sb, 0, 0, 2, 512))
                nc.vector.tensor_add(
                    rows_ap(tmp, 2048, 512, 6, 512),
                    rows_ap(v_od, PADL - 1 - 5, 2, 6, 512),
                    rows_ap(u_sb, 0, 0, 6, 512))
                tmp_sb[fc] = tmp
                if emit_gelu:
                    stage_d_gelu(fc)

            def stage_d_combine(fc):
                tmp = tmp_sb[fc]
                q = q1pool.tile([128, NOFF * 512], bf16, tag="q")
                nc.vector.tensor_mul(q[:], tmp[:], cw_bc[:])

                # pairwise tree-sum of the 10 weighted slices, then w-scale
                t1 = qpool.tile([128, 2560], bf16, tag="t1")
                nc.vector.tensor_add(t1[:], q[:, 0:2560], q[:, 2560:5120])
                t2 = qpool.tile([128, 1024], bf16, tag="t2")
                nc.vector.tensor_add(t2[:], t1[:, 0:1024], t1[:, 1024:2048])
                t3 = qpool.tile([128, 512], bf16, tag="t3")
                nc.vector.tensor_add(t3[:], t2[:, 0:512], t2[:, 512:1024])
                t4 = qpool.tile([128, 512], bf16, tag="t4")
                nc.vector.tensor_add(t4[:], t3[:], t1[:, 2048:2560])
                g_t = gpool.tile([128, 512], bf16, tag=f"g{fc}")
                nc.vector.tensor_mul(
                    g_t[:], t4[:],
                    w_bc_all[:, (fc // 2) * 512:(fc // 2) * 512 + 512])
                g_sb[fc] = g_t

            w2_sb = [None] * NFC

            def load_w2(j):
                t = w2pool.tile([128, D], bf16, tag=f"w2_{j}")
                nc.sync.dma_start(t[:], w2[j])
                w2_sb[j] = t

            def blk_mm(d_ps, blk, fc, start, stop=False):
                tci, dh = blk // 2, blk % 2
                nc.tensor.matmul(
                    d_ps[:],
                    g_sb[fc][:, tci * 128:(tci + 1) * 128],
                    w2_sb[fc][:, dh * 512:(dh + 1) * 512],
                    start=start, stop=stop)

            def blk_b2_mm(d_ps, blk):
                tci, dh = blk // 2, blk % 2
                nc.tensor.matmul(
                    d_ps[:],
                    wT_bf[:, tci * 128:(tci + 1) * 128],
                    b2_sb[:, dh * 512:(dh + 1) * 512],
                    start=False, stop=True)

            def out_dma(o_sb, blk):
                tci, dh = blk // 2, blk % 2
                nc.sync.dma_start(
                    out[tci * 128:(tci + 1) * 128,
                        dh * 512:(dh + 1) * 512], o_sb[:])

            # -- delta groups.  Held groups (blocks 0..2 on "s" banks, and
            # preopened blocks 3..6 on freed "m" banks) accumulate fc matmuls
            # per combine and close with b2 + a per-token 1/den scale.
            open_ps = {}

            def grp_open(blk, g_lo, g_hi, pool):
                d_ps = pool.tile([128, 512],
                                 mybir.dt.float32, tag="m" if pool is psb
                                 else "s", name=f"dps{blk}")
                for fc in range(g_lo, g_hi + 1):
                    blk_mm(d_ps, blk, fc, start=(fc == g_lo))
                open_ps[blk] = d_ps

            def grp_extend(blk, fc):
                blk_mm(open_ps[blk], blk, fc, start=False)

            def grp_close_direct(blk):
                """For groups that accumulated all of fc 0..15."""
                tci = blk // 2
                d_ps = open_ps[blk]
                blk_b2_mm(d_ps, blk)
                o_sb = opool.tile([128, 512], fp32, tag="o")
                nc.scalar.mul(o_sb[:], d_ps[:], rdenT[:, tci:tci + 1])
                out_dma(o_sb, blk)

            d_part = {}

            def stage_e1(blk):
                """fc 0..7 partial for blocks 3..7 (one pss bank transient)."""
                tci = blk // 2
                d_ps = pss.tile([128, 512], fp32, tag="s")
                for fc in range(8):
                    blk_mm(d_ps, blk, fc, start=(fc == 0), stop=(fc == 7))
                p_t = partpool.tile([128, 512], bf16, tag=f"p{blk}")
                nc.scalar.mul(p_t[:], d_ps[:], rdenT[:, tci:tci + 1])
                d_part[blk] = p_t

            def grp_close_merge(blk):
                """For groups that accumulated fc 8..15: merge with the E1
                partial via one scalar_tensor_tensor."""
                tci = blk // 2
                d_ps = open_ps[blk]
                blk_b2_mm(d_ps, blk)
                o_sb = opool.tile([128, 512], fp32, tag="o")
                nc.vector.scalar_tensor_tensor(
                    o_sb[:], d_ps[:], rdenT[:, tci:tci + 1], d_part[blk][:],
                    op0=OP.mult, op1=OP.add)
                out_dma(o_sb, blk)

            # ---- emission schedule ----
            phase_gram(0)
            phase_gram(1)
            stage_d_mm(0, w1_pre=w1_first, emit_gelu=False)
            phase_gram(2)
            phase_gram(3)
            phase_b1()
            phase_b2a_pre()
            phase_b2a_post()
            phase_b2b_early()
            stage_d_gelu(0)
            for fc in range(1, RUNWAY):
                stage_d_mm(fc)
            for j in range(NFC):            # combine index
                jj = j + RUNWAY
                if jj < NFC:
                    stage_d_mm(jj)
                    if 4 <= jj <= 11:
                        load_w2(2 * (jj - 4))
                        load_w2(2 * (jj - 4) + 1)
                stage_d_combine(j)
                if j == 1:
                    phase_b2b_late()
                if 7 <= j <= 10:
                    stage_e1(j - 4)         # blocks 3..6
                if j == 11:
                    stage_e1(7)
                    grp_open(0, 0, 11, pss)         # held, g0..11
                if j == 12:
                    grp_extend(0, 12)
                    grp_open(1, 0, 12, pss)         # held, g0..12
                    for blk in range(3, 7):         # preopens on freed m
                        grp_open(blk, 8, 12, psb)
                if j == 13:
                    for blk in (0, 1, 3, 4, 5, 6):
                        grp_extend(blk, 13)
                    grp_open(2, 0, 13, pss)         # held, g0..13
                if j == 14:
                    for blk in (0, 1, 2, 3, 4, 5, 6):
                        grp_extend(blk, 14)
            # tail: one g15 + b2 per open group, then block 7 full
            for blk in (0, 1, 2, 3, 4, 5, 6):
                grp_extend(blk, 15)
            grp_close_direct(0)
            grp_open(7, 8, 15, pss)
            grp_close_direct(1)
            grp_close_direct(2)
            for blk in (3, 4, 5, 6):
                grp_close_merge(blk)
            grp_close_merge(7)

    nc.compile()
    return nc


def _prep_shards(h_L, mask_flags, Wr, br, W1, b1, W2, b2):
    """Host-side shard construction (numpy only; cheap vs device work)."""
    f32 = np.float32
    h_L = np.asarray(h_L, f32)
    mask = np.asarray(mask_flags)
    Wr = np.asarray(Wr, f32)
    W1 = np.asarray(W1, f32)
    W2 = np.asarray(W2, f32)
    br = np.asarray(br, f32)
    b1 = np.asarray(b1, f32)
    b2 = np.asarray(b2, f32)

    # shared (replicated) weight blocks
    w1a = np.ascontiguousarray(
        W1[:, :D, :].transpose(1, 0, 2).reshape(D, F)
        .reshape(NKC, 128, NFC, 128).transpose(2, 1, 0, 3)
        .reshape(NFC, 128, D)).astype(BF16)
    w1b = np.ascontiguousarray(
        W1[:, D:, :].transpose(1, 0, 2).reshape(D, F)
        .reshape(NKC, 128, NFC, 128).transpose(2, 1, 0, 3)
        .reshape(NFC, 128, D)).astype(BF16)
    w1ab = np.concatenate([w1b, w1a], axis=2)        # [NFC, 128, 2D]
    w2p = np.ascontiguousarray(
        W2.reshape(F, D).reshape(NFC, 128, D)).astype(BF16)
    # packed wr: [128, NKC*K]
    wrP = np.ascontiguousarray(
        Wr.reshape(NKC, 128, K).transpose(1, 0, 2).reshape(128, NKC * K)
    ).astype(BF16)
    # packed consts: ident | br_bc | b1s  -> [128, 176] fp32
    br_bc = np.tile(np.broadcast_to(br[None, :], (128, K)), (1, NTC)).astype(f32)
    b1s = np.ascontiguousarray(b1.reshape(F).reshape(NFC, 128).T)
    cP = np.concatenate([np.eye(128, dtype=f32), br_bc, b1s], axis=1)
    # packed b2 + ones column: [NOFF, D+1] bf16
    b2o = np.zeros((NOFF, D + 2), BF16)
    b2o[:K, :D] = b2.astype(BF16)
    b2o[:, D] = 1.0

    offs = np.array(OFF_ORDER, np.int64)
    in_maps = []
    outs_meta = []
    per_batch = L // (NCORES // B)          # 512 tokens, 4 shards per batch
    for c in range(NCORES):
        b = c // (NCORES // B)
        t0 = (c % (NCORES // B)) * per_batch
        # padded, transposed h slice  [D, TP] -> packed [128, NKC*TP]
        hpad = np.zeros((TP, D), f32)
        lo = t0 - PADL
        hi = t0 + T + PADL
        slo, shi = max(lo, 0), min(hi, L)
        hpad[slo - lo:shi - lo] = h_L[b, slo:shi]
        hTa = np.ascontiguousarray(hpad.T).astype(BF16)          # [D, TP]
        hP = np.ascontiguousarray(
            hTa.reshape(NKC, 128, TP).transpose(1, 0, 2)
            .reshape(128, NKC * TP))

        # validity per (token, offset-order) -> [NOFF, T]
        tok = t0 + np.arange(T)
        nbr = tok[:, None] + offs[None, :]
        inb = (nbr >= 0) & (nbr < L)
        nbrc = np.clip(nbr, 0, L - 1)
        is_m = (mask[b] == 1)
        val = (inb & is_m[tok][:, None] & (~is_m[nbrc])).astype(f32)
        valT = np.ascontiguousarray(val.T)            # [NOFF, T]
        vtok = np.ascontiguousarray(
            val.reshape(NTC, 128, NOFF).transpose(1, 0, 2)
            .reshape(128, NTC * NOFF))
        cPc = np.concatenate([cP, vtok], axis=1)
        in_maps.append({
            "hP": hP, "cP": cPc, "wrP": wrP, "validT": valT, "b2o": b2o,
            "w1ab": w1ab, "w2": w2p,
        })
        outs_meta.append((b, t0))
    return in_maps, outs_meta


def kernel(**inputs):
    assert int(inputs["range_r"]) == R
    if "nc" not in _CACHE:
        _CACHE["nc"] = _build_graph()
    nc = _CACHE["nc"]
    in_maps, outs_meta = _prep_shards(
        inputs["h_L"], inputs["mask_flags"], inputs["Wr"], inputs["br"],
        inputs["W1"], inputs["b1"], inputs["W2"], inputs["b2"])
    res = run_bass_kernel_spmd(nc, in_maps, core_ids=list(range(NCORES)))
    out = np.zeros((B, L, D), np.float32)
    for c, (b, t0) in enumerate(outs_meta):
        out[b, t0:t0 + T] = res.results[c]["out"]
    return out


# revision 15
# speedup vs baseline: 1.0234x; 1.0116x over previous
"""Trainium2 Bass kernel for nn_AMIPRouterInference (gnn_message_passing).

Strategy
--------
Algebraic restructure of the reference (~515 GFLOP -> ~52 GFLOP):
  * cond @ W1 splits into h_anc @ W1a + h_ctr @ W1b, each computed once per
    token (not once per window pair):  u = h @ W1b, v = h @ W1a.
  * The attention combine over the +-r window commutes with the W2 matmul:
    g = sum_n cw_n * gelu(v[l+off_n] + u[l]);  delta = (w * g) @ W2 + w @ b2.

Sharding: pure data-parallel over the B*L = 4096 tokens -> 512 tokens/core on
8 cores; the +-5 halo is baked into each core's input shard on the host, so no
collectives are needed.

Per-core layout: features-on-partitions (u/v as 16 chunks of [128, tokens]) so
window shifts along tokens are free-axis SBUF slices.  Even/odd phase copies of
v keep the bf16 DVE 2x alignment for shifted adds.

Key engine facts this schedule is built around:
  * DVE is the bottleneck engine (~165us of tensor_tensor at bf16 2x).
    Batched multi-row-AP adds keep the 2x packing when every row start is
    4B-aligned (hardware-verified).
  * PE clock is HAM-gated: 1.2 GHz cold, 2.4 GHz after ~3.4us of sustained
    activity; any >3.4us idle window re-throttles.  The delta-stage matmuls
    are paced per-combine through the back half so the post-combine(15)
    tail is only ~23 matmuls.
  * DMA issue costs ~650ns per descriptor on the in-order sync queue, so
    startup inputs are packed host-side into 5 large contiguous transfers.
  * A 4-fc emission runway (d_mm 0..3 before combine 0) gives the DVE queue
    adds-work to chew while the exp->broadcast round trip for cw lands;
    combines then trail d_mm by 4 fc for the rest of the kernel, which also
    keeps ACT's gelu well ahead of the combine that consumes it.
"""

import sys

for _p in ("/opt/trn_rl_repo", "/root/.axon_site/_ro/trn_rl_repo"):
    if _p not in sys.path:
        sys.path.append(_p)

import numpy as np
import ml_dtypes

import bass_rust
import concourse.bacc as bacc
import concourse.mybir as mybir
import concourse.tile as tile
from concourse.bass_utils import run_bass_kernel_spmd

BF16 = ml_dtypes.bfloat16

# Problem constants (hardcoded per spec).
B, L, D = 2, 2048, 1024
K, D4, R = 8, 256, 5
NCORES = 8
T = (B * L) // NCORES          # tokens per core = 512
PADL = 16                      # left pad of the per-core token window
TP = T + 2 * PADL              # padded width = 544
NOFF = 2 * R                   # 10 window offsets
F = K * D4                     # 2048 fused expert features
NFC = F // 128                 # 16 feature chunks
NKC = D // 128                 # 8 contraction chunks
NTC = T // 128                 # 4 token tiles per core

# Offset processing order: even offsets first (read from v_even), then odd
# (read from v_odd, which holds v shifted left by one token).  Within each
# phase every slice start is an even element index -> 4-byte aligned, which
# keeps the DVE's bf16 2x packing for the batched multi-row adds.
OFF_ORDER = [-4, -2, 2, 4, -5, -3, -1, 1, 3, 5]

RUNWAY = 4                     # d_mm emitted this many fc ahead of combine

_SIM_SAFE_GELU = False         # CoreSim lacks Gelu; swap in Tanh for sim runs

_CACHE = {}


def _build_graph():
    fp32 = mybir.dt.float32
    bf16 = mybir.dt.bfloat16

    nc = bacc.Bacc("TRN2", target_bir_lowering=False, debug=False,
                   num_devices=NCORES)

    # ---- DRAM parameters (per-core shards; same shapes on every core).
    # Startup tensors are host-packed so each is ONE contiguous DMA.
    hP = nc.dram_tensor("hP", [128, NKC * TP], bf16, kind="ExternalInput")
    cP = nc.dram_tensor("cP", [128, 216], fp32, kind="ExternalInput")
    wrP = nc.dram_tensor("wrP", [128, NKC * K], bf16, kind="ExternalInput")
    validT = nc.dram_tensor("validT", [NOFF, T], fp32, kind="ExternalInput")
    b2o = nc.dram_tensor("b2o", [NOFF, D + 2], bf16, kind="ExternalInput")
    w1ab = nc.dram_tensor("w1ab", [NFC, 128, 2 * D], bf16,
                          kind="ExternalInput")
    w2 = nc.dram_tensor("w2", [NFC, 128, D], bf16, kind="ExternalInput")
    out = nc.dram_tensor("out", [T, D], fp32, kind="ExternalOutput")

    AF = mybir.ActivationFunctionType
    OP = mybir.AluOpType

    def bc_ap(tile_, inner_rep, ncols):
        """[128, ncols] tile viewed as [128, ncols, inner_rep] via a step-0
        innermost dim (per-partition broadcast along the replicated axis)."""
        return bass_rust.AP(
            tensor=tile_[:].tensor, offset=0,
            ap=[[ncols, 128], [1, ncols], [0, inner_rep]])

    def rows_ap(tile_, off, ostep, ocnt, icnt):
        """Multi-row free AP: ocnt rows of icnt step-1 elements, row starts
        off, off+ostep, ...  (all starts must be 4B-aligned for bf16 2x)."""
        return bass_rust.AP(
            tensor=tile_[:].tensor, offset=off,
            ap=[[tile_[:].shape[1], 128], [ostep, ocnt], [1, icnt]])

    with tile.TileContext(nc) as tc:
        with (
            tc.tile_pool(name="const", bufs=1) as cpool,
            tc.tile_pool(name="hpool", bufs=1) as hpool,
            tc.tile_pool(name="w2pool", bufs=1) as w2pool,
            tc.tile_pool(name="w1pool", bufs=4) as w1pool,
            tc.tile_pool(name="small", bufs=2) as spool,
            tc.tile_pool(name="persist", bufs=1) as ppool,
            tc.tile_pool(name="uv", bufs=4) as uvpool,
            tc.tile_pool(name="big", bufs=RUNWAY + 1) as bigpool,
            tc.tile_pool(name="qbuf", bufs=1) as q1pool,
            tc.tile_pool(name="tbuf", bufs=1) as qpool,
            tc.tile_pool(name="ppart", bufs=1) as partpool,
            tc.tile_pool(name="gout", bufs=1) as gpool,
            tc.tile_pool(name="opool", bufs=4) as opool,
            tc.tile_pool(name="dram", bufs=1, space="DRAM") as dpool,
            tc.tile_pool(name="ps_big", bufs=4, space="PSUM") as psb,
            tc.tile_pool(name="ps_vb", bufs=1, space="PSUM") as psvb,
            tc.tile_pool(name="ps_small", bufs=3, space="PSUM") as pss,
            # PSUM budget (8 banks): psb "m" 4 (u/va double buffer; the 4
            # banks are reused for delta preopens once stage D ends),
            # psvb "vb" 1, pss "s" 3 (logits, transposes, den, E1
            # transients, and the 3 held delta groups).
        ):
            # ---------------- packed startup loads ----------------
            h_m = hpool.tile([128, NKC * TP], bf16, tag="h")
            nc.sync.dma_start(h_m[:], hP[:])
            c_m = cpool.tile([128, 216], fp32, tag="c")
            nc.sync.dma_start(c_m[:], cP[:])
            wr_m = cpool.tile([128, NKC * K], bf16, tag="wr")
            nc.sync.dma_start(wr_m[:], wrP[:])
            validT_sb = cpool.tile([NOFF, T], fp32, tag="validT")
            nc.sync.dma_start(validT_sb[:], validT[:])
            b2o_sb = cpool.tile([NOFF, D + 2], bf16, tag="b2o")
            nc.sync.dma_start(b2o_sb[:], b2o[:])

            h_sb = [h_m[:, kc * TP:(kc + 1) * TP] for kc in range(NKC)]
            ident_sb = c_m[:, 0:128]
            br_sb = c_m[:, 128:160]
            b1_sb = c_m[:, 160:176]
            vtok_sb = c_m[:, 176:216]
            wr_sb = [wr_m[:, kc * K:(kc + 1) * K] for kc in range(NKC)]
            b2_sb = b2o_sb[0:K, 0:D]
            ones10_sb = b2o_sb[:, D:D + 1]

            # fc0's W1 rides the (idle) Tensor-engine DMA queue so it
            # lands in parallel with the sync queue's h/const loads.
            w1_first = w1pool.tile([128, 2 * D], bf16, tag="w1")
            nc.scalar.dma_start(w1_first[:], w1ab[0])

            # Hoist both ACT table loads (exp + gelu sets, ~1.3us each) into
            # the h-DMA shadow via 1-column dummy activations.
            warm = spool.tile([1, 1], fp32, tag="warm")
            nc.scalar.activation(warm[:], ident_sb[0:1, 0:1], AF.Exp)

            # persistent transposed score & router weights (bf16)
            cwT_bf = ppool.tile([NOFF, T], bf16, tag="cwT")
            wT_bf = ppool.tile([K, T], bf16, tag="wT")
            cw_bc = gpool.tile([128, NOFF * 512], bf16, tag="cw_bc")
            w_bc_all = gpool.tile([128, K * 512], bf16, tag="w_bc_all")

            # ------------- stage A/B/C: scores, cw, router w -------------
            # Phase A: gram/router matmuls; each tile's diagonal extraction
            # is emitted right behind its gram so DVE starts ASAP; the logit
            # evac follows immediately so only one "s" bank is held per tile.
            s_all = spool.tile([128, NTC * NOFF], fp32, tag="s_all")
            junk = spool.tile([128, 128], fp32, tag="junk")
            lg_all = spool.tile([128, NTC * K], fp32, tag="lg_all")

            def phase_gram(tci):
                c0 = PADL + tci * 128
                g_ps = psb.tile([128, 512], fp32, tag="m")
                lg_ps = pss.tile([128, K], fp32, tag="s")
                for kc in range(NKC):
                    st = (kc == 0)
                    sp = (kc == NKC - 1)
                    nc.tensor.matmul(g_ps[:, :138],
                                     h_sb[kc][:, c0:c0 + 128],
                                     h_sb[kc][:, c0 - 5:c0 + 133],
                                     start=st, stop=sp)
                    nc.tensor.matmul(lg_ps[:],
                                     h_sb[kc][:, c0:c0 + 128],
                                     wr_sb[kc],
                                     start=st, stop=sp)
                for n, off in enumerate(OFF_ORDER):
                    nc.vector.affine_mul_reduce(
                        junk[:], s_all[:, tci * NOFF + n:tci * NOFF + n + 1],
                        g_ps[:, off + 5:off + 5 + 128], ident_sb,
                        1.0 / 32.0, 0.0)
                nc.scalar.copy(lg_all[:, tci * K:(tci + 1) * K], lg_ps[:])

            # Phase B1: transpose scores to [NOFF, T], then broadcast the
            # UNNORMALIZED ev = exp(s)*valid right away.  The softmax 1/den
            # lands at the very end as a per-token (=per-partition) scale on
            # the delta close.  No max-shift is needed: scores are O(1) so
            # exp() cannot over/underflow.
            sT = ppool.tile([NOFF, T], fp32, tag="sT")

            def phase_b1():
                for tci in range(NTC):
                    sT_ps = pss.tile([NOFF, 128], fp32, tag="s")
                    nc.tensor.transpose(sT_ps[:],
                                        s_all[:, tci * NOFF:(tci + 1) * NOFF],
                                        ident_sb)
                    nc.scalar.copy(sT[:, tci * 128:(tci + 1) * 128], sT_ps[:])
                evT = ppool.tile([NOFF, T], fp32, tag="evT")
                nc.scalar.activation(evT[:], sT[:], AF.Exp)
                nc.vector.tensor_mul(cwT_bf[:], evT[:], validT_sb[:])
                cw_dram = dpool.tile([1, NOFF * T], bf16, tag="cw_dram")
                nc.scalar.dma_start(cw_dram[:], cwT_bf[:])
                nc.scalar.dma_start(cw_bc[:],
                                    cw_dram[:].partition_broadcast(128))

            rdenT = ppool.tile([128, NTC], fp32, tag="rdenT")
            fT = ppool.tile([128, NTC], fp32, tag="fT")
            wplT_bf = ppool.tile([K, T], bf16, tag="wplT")
            w_pl = ppool.tile([128, NTC * K], fp32, tag="w_pl")

            we = spool.tile([128, NTC * K], fp32, tag="we")

            def phase_b2a_pre():
                """Router softmax up to the exp."""
                nc.vector.tensor_add(lg_all[:], lg_all[:], br_sb)
                wmx = spool.tile([128, NTC], fp32, tag="wmx")
                lg3 = bass_rust.AP(tensor=lg_all[:].tensor, offset=0,
                                   ap=[[NTC * K, 128], [K, NTC], [1, K]])
                nc.vector.reduce_max(wmx[:], lg3, mybir.AxisListType.X)
                nc.vector.tensor_sub(we[:], lg_all[:], bc_ap(wmx, K, NTC))
                nc.scalar.activation(we[:], we[:], AF.Exp)

            def phase_b2a_post():
                """Post-exp half: w softmax, transpose, broadcast.  Emitted
                before the first gelu so the w_dram round trip isn't queued
                behind 5.5us ACT gelus."""
                wsum = spool.tile([128, NTC], fp32, tag="wsum")
                we3 = bass_rust.AP(tensor=we[:].tensor, offset=0,
                                   ap=[[NTC * K, 128], [K, NTC], [1, K]])
                nc.vector.reduce_sum(wsum[:], we3, mybir.AxisListType.X)
                rws = spool.tile([128, NTC], fp32, tag="rws")
                nc.vector.reciprocal(rws[:], wsum[:])
                nc.vector.tensor_mul(w_pl[:], we[:], bc_ap(rws, K, NTC))
                for tci in range(NTC):
                    wpT_ps = pss.tile([K, 128], fp32, tag="s")
                    nc.tensor.transpose(wpT_ps[:],
                                        w_pl[:, tci * K:(tci + 1) * K],
                                        ident_sb)
                    nc.scalar.copy(wplT_bf[:, tci * 128:(tci + 1) * 128],
                                   wpT_ps[:])
                w_dram = dpool.tile([1, K * T], bf16, tag="w_dram")
                nc.scalar.dma_start(w_dram[:], wplT_bf[:])
                nc.scalar.dma_start(w_bc_all[:],
                                    w_dram[:].partition_broadcast(128))

            weff3 = spool.tile([128, NTC * K], fp32, tag="weff3")

            def phase_b2b_early():
                """Denominator per token, computed token-major entirely on
                DVE (+1 tiny ACT exp) so no PE matmul or DMA round trip sits
                on the early critical path.  The reciprocal runs on the
                [128, NTC] layout -- a [1, T] reciprocal would serialize 512
                8-cycle divides on one partition (~4us)."""
                evm = spool.tile([128, NTC * NOFF], fp32, tag="evm")
                nc.scalar.activation(evm[:], s_all[:], AF.Exp)
                nc.vector.tensor_mul(evm[:], evm[:], vtok_sb)
                ev3 = bass_rust.AP(tensor=evm[:].tensor, offset=0,
                                   ap=[[NTC * NOFF, 128], [NOFF, NTC],
                                       [1, NOFF]])
                nc.vector.reduce_sum(fT[:], ev3, mybir.AxisListType.X)
                dene_t = ppool.tile([128, NTC], fp32, tag="dene_t")
                nc.vector.tensor_scalar_add(dene_t[:], fT[:], 1e-30)
                nc.vector.reciprocal(rdenT[:], dene_t[:])
                # b2-path weights: w * raw_den (so the final 1/den scale on
                # the delta close reproduces w * sum_cw exactly)
                for tci in range(NTC):
                    nc.vector.tensor_scalar_mul(
                        weff3[:, tci * K:(tci + 1) * K],
                        w_pl[:, tci * K:(tci + 1) * K], fT[:, tci:tci + 1])

            def phase_b2b_late():
                """Tiny weff transposes; deferred so they never head-of-line
                block the PE queue while waiting on weff3."""
                for tci in range(NTC):
                    weT_ps = pss.tile([K, 128], fp32, tag="s")
                    nc.tensor.transpose(weT_ps[:],
                                        weff3[:, tci * K:(tci + 1) * K],
                                        ident_sb)
                    nc.scalar.copy(wT_bf[:, tci * 128:(tci + 1) * 128],
                                   weT_ps[:])

            # ------------- stage D: u/v matmuls + gelu combine -------------
            g_sb = [None] * NFC
            tmp_sb = [None] * NFC

            def stage_d_gelu(fc):
                tmp = tmp_sb[fc]
                nc.scalar.activation(tmp[:], tmp[:],
                                     AF.Tanh if _SIM_SAFE_GELU else AF.Gelu)

            def stage_d_mm(fc, w1_pre=None, emit_gelu=True):
                if w1_pre is None:
                    w1_t = w1pool.tile([128, 2 * D], bf16, tag="w1")
                    nc.sync.dma_start(w1_t[:], w1ab[fc])
                else:
                    w1_t = w1_pre

                u_ps = psb.tile([128, 512], fp32, tag="m")
                va_ps = psb.tile([128, 512], fp32, tag="m")
                vb_ps = psvb.tile([128, 48], fp32, tag="vb")
                for kc in range(NKC):
                    st = (kc == 0)
                    sp = (kc == NKC - 1)
                    lhs_b = w1_t[:, kc * 128:(kc + 1) * 128]
                    lhs_a = w1_t[:, D + kc * 128:D + (kc + 1) * 128]
                    nc.tensor.matmul(u_ps[:], lhs_b,
                                     h_sb[kc][:, PADL:PADL + 512],
                                     start=st, stop=sp)
                    nc.tensor.matmul(va_ps[:], lhs_a,
                                     h_sb[kc][:, 0:512],
                                     start=st, stop=sp)
                    nc.tensor.matmul(vb_ps[:], lhs_a,
                                     h_sb[kc][:, 496:544],
                                     start=st, stop=sp)

                u_sb = uvpool.tile([128, 512], bf16, tag="u")
                nc.scalar.activation(u_sb[:], u_ps[:], AF.Identity,
                                     bias=b1_sb[:, fc:fc + 1])
                v_ev = uvpool.tile([128, TP], bf16, tag="v_ev")
                nc.scalar.copy(v_ev[:, 0:512], va_ps[:])
                nc.scalar.copy(v_ev[:, 512:544], vb_ps[:, 16:48])
                # odd phase built straight from PSUM (keeps DMA out of the
                # critical chain)
                v_od = uvpool.tile([128, TP], bf16, tag="v_od")
                nc.scalar.copy(v_od[:, 0:511], va_ps[:, 1:512])
                nc.scalar.copy(v_od[:, 511:543], vb_ps[:, 16:48])

                tmp = bigpool.tile([128, NOFF * 512], bf16, tag="tmp")
                # Batched shifted adds: every row start is an even element
                # index (4B-aligned), so the multi-row APs keep the DVE's
                # bf16 2x packing (hardware-verified: 6-row 1752ns vs
                # 6x418ns single-row).  Layout matches OFF_ORDER:
                #   [0:1024)    offs -4,-2    from v_ev
                #   [1024:2048) offs 2,4      from v_ev
                #   [2048:5120) offs -5..5 odd from v_od
                nc.vector.tensor_add(
                    rows_ap(tmp, 0, 512, 2, 512),
                    rows_ap(v_ev, PADL - 4, 2, 2, 512),
                    rows_ap(u_sb, 0, 0, 2, 512))
                nc.vector.tensor_add(
                    rows_ap(tmp, 1024, 512, 2, 512),
                    rows_ap(v_ev, PADL + 2, 2, 2, 512),
                    rows_ap(u_sb, 0, 0, 2, 512))
                nc.vector.tensor_add(
                    rows_ap(tmp, 2048, 512, 6, 512),
                    rows_ap(v_od, PADL - 1 - 5, 2, 6, 512),
                    rows_ap(u_sb, 0, 0, 6, 512))
                tmp_sb[fc] = tmp
                if emit_gelu:
                    stage_d_gelu(fc)

            def stage_d_combine(fc):
                tmp = tmp_sb[fc]
                q = q1pool.tile([128, NOFF * 512], bf16, tag="q")
                nc.vector.tensor_mul(q[:], tmp[:], cw_bc[:])

                # pairwise tree-sum of the 10 weighted slices, then w-scale
                t1 = qpool.tile([128, 2560], bf16, tag="t1")
                nc.vector.tensor_add(t1[:], q[:, 0:2560], q[:, 2560:5120])
                t2 = qpool.tile([128, 1024], bf16, tag="t2")
                nc.vector.tensor_add(t2[:], t1[:, 0:1024], t1[:, 1024:2048])
                t3 = qpool.tile([128, 512], bf16, tag="t3")
                nc.vector.tensor_add(t3[:], t2[:, 0:512], t2[:, 512:1024])
                t4 = qpool.tile([128, 512], bf16, tag="t4")
                nc.vector.tensor_add(t4[:], t3[:], t1[:, 2048:2560])
                g_t = gpool.tile([128, 512], bf16, tag=f"g{fc}")
                nc.vector.tensor_mul(
                    g_t[:], t4[:],
                    w_bc_all[:, (fc // 2) * 512:(fc // 2) * 512 + 512])
                g_sb[fc] = g_t

            w2_sb = [None] * NFC

            def load_w2(j):
                t = w2pool.tile([128, D], bf16, tag=f"w2_{j}")
                nc.sync.dma_start(t[:], w2[j])
                w2_sb[j] = t

            def blk_mm(d_ps, blk, fc, start, stop=False):
                tci, dh = blk // 2, blk % 2
                nc.tensor.matmul(
                    d_ps[:],
                    g_sb[fc][:, tci * 128:(tci + 1) * 128],
                    w2_sb[fc][:, dh * 512:(dh + 1) * 512],
                    start=start, stop=stop)

            def blk_b2_mm(d_ps, blk):
                tci, dh = blk // 2, blk % 2
                nc.tensor.matmul(
                    d_ps[:],
                    wT_bf[:, tci * 128:(tci + 1) * 128],
                    b2_sb[:, dh * 512:(dh + 1) * 512],
                    start=False, stop=True)

            def out_dma(o_sb, blk):
                tci, dh = blk // 2, blk % 2
                nc.sync.dma_start(
                    out[tci * 128:(tci + 1) * 128,
                        dh * 512:(dh + 1) * 512], o_sb[:])

            # -- delta groups.  Held groups (blocks 0..2 on "s" banks, and
            # preopened blocks 3..6 on freed "m" banks) accumulate fc matmuls
            # per combine and close with b2 + a per-token 1/den scale.
            open_ps = {}

            def grp_open(blk, g_lo, g_hi, pool):
                d_ps = pool.tile([128, 512],
                                 mybir.dt.float32, tag="m" if pool is psb
                                 else "s", name=f"dps{blk}")
                for fc in range(g_lo, g_hi + 1):
                    blk_mm(d_ps, blk, fc, start=(fc == g_lo))
                open_ps[blk] = d_ps

            def grp_extend(blk, fc):
                blk_mm(open_ps[blk], blk, fc, start=False)

            def grp_close_direct(blk):
                """For groups that accumulated all of fc 0..15."""
                tci = blk // 2
                d_ps = open_ps[blk]
                blk_b2_mm(d_ps, blk)
                o_sb = opool.tile([128, 512], fp32, tag="o")
                nc.scalar.mul(o_sb[:], d_ps[:], rdenT[:, tci:tci + 1])
                out_dma(o_sb, blk)

            d_part = {}

            def stage_e1(blk):
                """fc 0..7 partial for blocks 3..7 (one pss bank transient)."""
                tci = blk // 2
                d_ps = pss.tile([128, 512], fp32, tag="s")
                for fc in range(8):
                    blk_mm(d_ps, blk, fc, start=(fc == 0), stop=(fc == 7))
                p_t = partpool.tile([128, 512], bf16, tag=f"p{blk}")
                nc.scalar.mul(p_t[:], d_ps[:], rdenT[:, tci:tci + 1])
                d_part[blk] = p_t

            def grp_close_merge(blk):
                """For groups that accumulated fc 8..15: merge with the E1
                partial via one scalar_tensor_tensor."""
                tci = blk // 2
                d_ps = open_ps[blk]
                blk_b2_mm(d_ps, blk)
                o_sb = opool.tile([128, 512], fp32, tag="o")
                nc.vector.scalar_tensor_tensor(
                    o_sb[:], d_ps[:], rdenT[:, tci:tci + 1], d_part[blk][:],
                    op0=OP.mult, op1=OP.add)
                out_dma(o_sb, blk)

            # ---- emission schedule ----
            phase_gram(0)
            phase_gram(1)
            stage_d_mm(0, w1_pre=w1_first, emit_gelu=False)
            phase_gram(2)
            phase_gram(3)
            phase_b1()
            phase_b2a_pre()
            phase_b2a_post()
            phase_b2b_early()
            stage_d_gelu(0)
            for fc in range(1, RUNWAY):
                stage_d_mm(fc)
            for j in range(NFC):            # combine index
                jj = j + RUNWAY
                if jj < NFC:
                    stage_d_mm(jj)
                    if 4 <= jj <= 11:
                        load_w2(2 * (jj - 4))
                        load_w2(2 * (jj - 4) + 1)
                stage_d_combine(j)
                if j == 1:
                    phase_b2b_late()
                if 7 <= j <= 10:
                    stage_e1(j - 4)         # blocks 3..6
                if j == 11:
                    stage_e1(7)
                    grp_open(0, 0, 11, pss)         # held, g0..11
                if j == 12:
                    grp_extend(0, 12)
                    grp_open(1, 0, 12, pss)         # held, g0..12
                    for blk in range(3, 7):         # preopens on freed m
                        grp_open(blk, 8, 12, psb)
                if j == 13:
                    for blk in (0, 1, 3, 4, 5, 6):
                        grp_extend(blk, 13)
                    grp_open(2, 0, 13, pss)         # held, g0..13
                if j == 14:
                    for blk in (0, 1, 2, 3, 4, 5, 6):
                        grp_extend(blk, 14)
            # tail: one g15 + b2 per open group, then block 7 full
            for blk in (0, 1, 2, 3, 4, 5, 6):
                grp_extend(blk, 15)
            grp_close_direct(0)
            grp_open(7, 8, 15, pss)
            grp_close_direct(1)
            grp_close_direct(2)
            for blk in (3, 4, 5, 6):
                grp_close_merge(blk)
            grp_close_merge(7)

    nc.compile()
    return nc


def _prep_shards(h_L, mask_flags, Wr, br, W1, b1, W2, b2):
    """Host-side shard construction (numpy only; cheap vs device work)."""
    f32 = np.float32
    h_L = np.asarray(h_L, f32)
    mask = np.asarray(mask_flags)
    Wr = np.asarray(Wr, f32)
    W1 = np.asarray(W1, f32)
    W2 = np.asarray(W2, f32)
    br = np.asarray(br, f32)
    b1 = np.asarray(b1, f32)
    b2 = np.asarray(b2, f32)

    # shared (replicated) weight blocks
    w1a = np.ascontiguousarray(
        W1[:, :D, :].transpose(1, 0, 2).reshape(D, F)
        .reshape(NKC, 128, NFC, 128).transpose(2, 1, 0, 3)
        .reshape(NFC, 128, D)).astype(BF16)
    w1b = np.ascontiguousarray(
        W1[:, D:, :].transpose(1, 0, 2).reshape(D, F)
        .reshape(NKC, 128, NFC, 128).transpose(2, 1, 0, 3)
        .reshape(NFC, 128, D)).astype(BF16)
    w1ab = np.concatenate([w1b, w1a], axis=2)        # [NFC, 128, 2D]
    w2p = np.ascontiguousarray(
        W2.reshape(F, D).reshape(NFC, 128, D)).astype(BF16)
    # packed wr: [128, NKC*K]
    wrP = np.ascontiguousarray(
        Wr.reshape(NKC, 128, K).transpose(1, 0, 2).reshape(128, NKC * K)
    ).astype(BF16)
    # packed consts: ident | br_bc | b1s  -> [128, 176] fp32
    br_bc = np.tile(np.broadcast_to(br[None, :], (128, K)), (1, NTC)).astype(f32)
    b1s = np.ascontiguousarray(b1.reshape(F).reshape(NFC, 128).T)
    cP = np.concatenate([np.eye(128, dtype=f32), br_bc, b1s], axis=1)
    # packed b2 + ones column: [NOFF, D+1] bf16
    b2o = np.zeros((NOFF, D + 2), BF16)
    b2o[:K, :D] = b2.astype(BF16)
    b2o[:, D] = 1.0

    offs = np.array(OFF_ORDER, np.int64)
    in_maps = []
    outs_meta = []
    per_batch = L // (NCORES // B)          # 512 tokens, 4 shards per batch
    for c in range(NCORES):
        b = c // (NCORES // B)
        t0 = (c % (NCORES // B)) * per_batch
        # padded, transposed h slice  [D, TP] -> packed [128, NKC*TP]
        hpad = np.zeros((TP, D), f32)
        lo = t0 - PADL
        hi = t0 + T + PADL
        slo, shi = max(lo, 0), min(hi, L)
        hpad[slo - lo:shi - lo] = h_L[b, slo:shi]
        hTa = np.ascontiguousarray(hpad.T).astype(BF16)          # [D, TP]
        hP = np.ascontiguousarray(
            hTa.reshape(NKC, 128, TP).transpose(1, 0, 2)
            .reshape(128, NKC * TP))

        # validity per (token, offset-order) -> [NOFF, T]
        tok = t0 + np.arange(T)
        nbr = tok[:, None] + offs[None, :]
        inb = (nbr >= 0) & (nbr < L)
        nbrc = np.clip(nbr, 0, L - 1)
        is_m = (mask[b] == 1)
        val = (inb & is_m[tok][:, None] & (~is_m[nbrc])).astype(f32)
        valT = np.ascontiguousarray(val.T)            # [NOFF, T]
        vtok = np.ascontiguousarray(
            val.reshape(NTC, 128, NOFF).transpose(1, 0, 2)
            .reshape(128, NTC * NOFF))
        cPc = np.concatenate([cP, vtok], axis=1)
        in_maps.append({
            "hP": hP, "cP": cPc, "wrP": wrP, "validT": valT, "b2o": b2o,
            "w1ab": w1ab, "w2": w2p,
        })
        outs_meta.append((b, t0))
    return in_maps, outs_meta


def kernel(**inputs):
    assert int(inputs["range_r"]) == R
    if "nc" not in _CACHE:
        _CACHE["nc"] = _build_graph()
    nc = _CACHE["nc"]
    in_maps, outs_meta = _prep_shards(
        inputs["h_L"], inputs["mask_flags"], inputs["Wr"], inputs["br"],
        inputs["W1"], inputs["b1"], inputs["W2"], inputs["b2"])
    res = run_bass_kernel_spmd(nc, in_maps, core_ids=list(range(NCORES)))
    out = np.zeros((B, L, D), np.float32)
    for c, (b, t0) in enumerate(outs_meta):
        out[b, t0:t0 + T] = res.results[c]["out"]
    return out
